# revision 1
# baseline (speedup 1.0000x reference)
"""Trainium2 Bass kernel for the CandidateFinder sparse-attention problem.

Computes, for each (batch, query) row, the first K_MAX=64 key indices whose
32-bit sign pattern exactly matches the query's in either of two dim groups
(dims 0:32, 32:64), padded with -1.

Approach (per core; 8 cores = 4 batches x 2 query halves):
  - signs s = 2*(x>0)-1 in bf16 (exact); per group S_g[q,j] = sum_d s_q s_k
    is an integer in [-32,32]; match <=> S_g == 32. (TensorE, K=34.)
  - two extra contraction rows add ramp(j) = (2048-j)*2^-13 (sum of two
    bf16-exact products), so S'_g = S_g + ramp is exact in fp32 PSUM and
    strictly decreasing in j for fixed S.
  - ScalarE evacuates group 2 as relu(S'_2 - 32) -> fp16 (matched positions
    give exactly (2048-j)*2^-13, fp16-exact and descending in j; rest 0);
    a fused DVE op evacuates group 1 and merges:
    val = max(S'_1 - 32, relu(S'_2 - 32)).
  - two pairwise fp16 max folds shrink the row 2048 -> 512 before the DVE
    `max` (hardware top-8, descending) extracts the first <=8 matching j;
    max never alters values, so survivors still encode j exactly. A fold
    loses a match only if two nonzeros share a fold group, which is
    detected exactly by sum conservation (accum_out sums are fp32-exact;
    sum(val) > sum(M2) iff some fold had two positives) and turns into a
    forced-positive 8th slot.
  - three 2-source ops decode the top-8 values to j / -1.
  - rows whose 8th candidate decodes as a real match (>=8 real matches, or
    the collision flag) are recomputed exactly on the host with numpy. With
    random normal inputs this never triggers: a match needs a 2^-32
    sign-pattern collision.

Self-contained: hardcodes shapes from the problem spec.
"""

import numpy as np

B = 4
L = 2048
D = 64
K_MAX = 64
N_CORES = 8
QSH = B * L // N_CORES  # 1024 queries per core
N_QT = QSH // 128       # 8 query tiles per core

_CACHE = {}


def _build_program(reps=1):
    from contextlib import ExitStack

    import concourse.bacc as bacc
    import concourse.mybir as mybir
    import concourse.tile as tile

    dt = mybir.dt
    Alu = mybir.AluOpType

    # Bacc (not raw Bass): its legalization passes split multi-sem waits,
    # which PE instructions can't carry (1 wait max per instruction).
    nc = bacc.Bacc("TRN2", target_bir_lowering=False, debug=False)
    qT_d = nc.declare_dram_parameter("qT", [D, QSH], dt.float32, isOutput=False)
    kT_d = nc.declare_dram_parameter("kT", [D, L], dt.float32, isOutput=False)
    ramp_d = nc.declare_dram_parameter("ramp", [2, L], dt.bfloat16, isOutput=False)
    out_d = nc.declare_dram_parameter("out", [QSH, K_MAX], dt.int32, isOutput=True)

    with tile.TileContext(nc) as tc, ExitStack() as ctx:
        consts = ctx.enter_context(tc.tile_pool(name="consts", bufs=1))
        vals = ctx.enter_context(tc.tile_pool(name="vals", bufs=3))
        outs = ctx.enter_context(tc.tile_pool(name="outs", bufs=1))
        psum = ctx.enter_context(tc.tile_pool(name="psum", bufs=2, space="PSUM"))

        # ---- load raw (transposed) inputs ----
        qraw = consts.tile([D, QSH], dt.float32)
        kraw = consts.tile([D, L], dt.float32)
        nc.sync.dma_start(qraw[:], qT_d[:])
        nc.sync.dma_start(kraw[:], kT_d[:])

        # per-partition bias constant for the relu evacuation
        bias32 = consts.tile([128, 1], dt.float32, tag="bias32")
        nc.vector.memset(bias32[:], -32.0)
        # decode constants (tiles so decode ops can be 2-source 1x-mode ops)
        c2048 = consts.tile([128, 64], dt.float32, tag="c2048")
        nc.vector.memset(c2048[:], 2048.0)
        z64 = consts.tile([128, 64], dt.float32, tag="z64")
        nc.vector.memset(z64[:], 0.0)
        # -1 padding for output columns 8..63
        pad56 = consts.tile([128, K_MAX - 8], dt.int32, tag="pad56")
        nc.vector.memset(pad56[:], -1)
        # all 8 query tiles' top-8 values, decoded in one shot at the end
        t8all = consts.tile([128, 64], dt.float16, tag="t8all")

        # ---- sign tiles (+ ramp rows) ----
        # QS[g]: [34, QSH]  rows 0:32 = signs of dims g*32:(g+1)*32,
        #                   rows 32/33 = 1.0 (ramp passthrough weights)
        # KS[g]: [34, L]    rows 0:32 = key signs, rows 32/33 = ramp terms
        QS = []
        KS = []
        # sign bias: sign(x - 1e-20) == 2*(x>0)-1 for every fp32 value the
        # randn inputs can take (smallest nonzero magnitude ~3e-7), and maps
        # x == 0.0 to -1 exactly like the reference's (x > 0).
        eps_b = consts.tile([64, 1], dt.float32, tag="eps_b")
        nc.vector.memset(eps_b[:], -1e-20)

        def sign_dve(dst, src):
            # s = ((x > 0)*2) - 1 in two DVE tensor_scalar passes (exact,
            # including x == 0 -> -1, matching the reference's (x > 0))
            nc.vector.tensor_scalar(
                out=dst, in0=src,
                scalar1=0.0, scalar2=2.0, op0=Alu.is_gt, op1=Alu.mult)
            nc.vector.tensor_scalar(
                out=dst, in0=dst, scalar1=-1.0, scalar2=None, op0=Alu.add)

        for g in range(2):
            qs = consts.tile([34, QSH], dt.bfloat16, tag=f"qs{g}")
            ks = consts.tile([34, L], dt.bfloat16, tag=f"ks{g}")
            lo, hi = g * 32, (g + 1) * 32
            # Sign prep gates the first matmuls: split it DVE (3 tensors,
            # exact is_gt path) / ACT (keys group 2, Sign(x - eps) which
            # equals 2*(x>0)-1 for every reachable fp32 randn value).
            if g == 0:
                # keys-h0 first so the first matmuls' rhs is ready earliest
                sign_dve(ks[0:32, 0:1024], kraw[lo:hi, 0:1024])
                sign_dve(qs[0:32, :], qraw[lo:hi, :])
                sign_dve(ks[0:32, 1024:2048], kraw[lo:hi, 1024:2048])
            else:
                sign_dve(qs[0:32, :], qraw[lo:hi, :])
                nc.scalar.activation(
                    ks[0:32, :], kraw[lo:hi, :],
                    mybir.ActivationFunctionType.Sign,
                    bias=eps_b[0:32, :], scale=1.0)
            nc.vector.memset(qs[32:34, :], 1.0)
            # ramp terms (host-precomputed bf16 constants) into rows 32/33
            nc.sync.dma_start(ks[32:34, :], ramp_d[:])
            QS.append(qs)
            KS.append(ks)

        # ---- main loop over query tiles ----
        # reps>1 repeats the whole body inside one NEFF (timing only).
        for t in [qt for _ in range(reps) for qt in range(N_QT)]:
            # ScalarE evacuates group 2 with relu(S'_2 - 32) -> fp16 (matched
            # positions give exactly (2048-j)*2^-13, everything else 0);
            # DVE then fuses group 1's evacuation with the merge:
            # val = (S'_1 - 32) max relu(S'_2 - 32) == relu(max(S'_1,S'_2)-32)
            # for the matched range, since all matched values are > 0.
            v2 = vals.tile([128, L], dt.float16, tag="v2")
            val = vals.tile([128, L], dt.float16, tag="val")
            sv = vals.tile([128, 2], dt.float32, tag="sv")
            for h in range(2):  # halves of the key axis
                p0 = psum.tile([128, 1024], dt.float32, tag="p0")
                p1 = psum.tile([128, 1024], dt.float32, tag="p1")
                for g, pg in enumerate((p0, p1)):
                    for n in range(2):
                        nc.tensor.matmul(
                            pg[:, n * 512:(n + 1) * 512],
                            QS[g][:, t * 128:(t + 1) * 128],
                            KS[g][:, h * 1024 + n * 512: h * 1024 + (n + 1) * 512],
                            start=True, stop=True)
                cols = slice(h * 1024, (h + 1) * 1024)
                nc.scalar.activation(
                    v2[:, cols], p1[:], mybir.ActivationFunctionType.Relu,
                    bias=bias32[:], scale=1.0)
                # accum_out gives sum(val half) for free (exact in fp32:
                # all values are multiples of 2^-13 bounded by 512)
                nc.vector.scalar_tensor_tensor(
                    out=val[:, cols], in0=p0[:], scalar=-32.0,
                    in1=v2[:, cols], op0=Alu.add, op1=Alu.max,
                    accum_out=sv[:, h:h + 1])

            # Shrink the top-8 scan 2048 -> 512 with two pairwise max folds.
            # A fold only loses information if both elements of a pair are
            # nonzero ("collision"); then sum(M2) < sum(val) strictly, which
            # the fp32-exact sums detect. max() never alters values, so the
            # surviving entries still encode j exactly.
            m1 = vals.tile([128, L // 2], dt.float16, tag="m1")
            nc.vector.tensor_tensor(
                out=m1[:], in0=val[:, 0:1024], in1=val[:, 1024:2048],
                op=Alu.max)
            m2 = vals.tile([128, L // 4], dt.float16, tag="m2")
            sm = vals.tile([128, 1], dt.float32, tag="sm")
            nc.vector.scalar_tensor_tensor(
                out=m2[:], in0=m1[:, 0:512], scalar=0.0,
                in1=m1[:, 512:1024], op0=Alu.add, op1=Alu.max,
                accum_out=sm[:])

            # top-8 values per query row, descending == first <=8 matches
            nc.vector.max(t8all[:, 8 * t:8 * t + 8], m2[:])

            # collision flag -> force slot 7 positive, which triggers the
            # same exact host fallback as the >8-matches case.
            svt = vals.tile([128, 1], dt.float32, tag="svt")
            nc.vector.tensor_tensor(
                out=svt[:], in0=sv[:, 0:1], in1=sv[:, 1:2], op=Alu.add)
            flag = vals.tile([128, 1], dt.float32, tag="flag")
            nc.vector.tensor_tensor(
                out=flag[:], in0=svt[:], in1=sm[:], op=Alu.is_gt)
            nc.vector.scalar_tensor_tensor(
                out=t8all[:, 8 * t + 7:8 * t + 8], in0=flag[:],
                scalar=2.0 ** -13, in1=t8all[:, 8 * t + 7:8 * t + 8],
                op0=Alu.mult, op1=Alu.max)

        # ---- decode all tiles at once ----
        # matched v = (2048-j)*2^-13 => u = 2048 - 8192*v = j in [0, 2047];
        # unmatched v = 0 => u = 2048 -> -1.
        u = outs.tile([128, 64], dt.float32, tag="u")
        nc.vector.scalar_tensor_tensor(
            out=u[:], in0=t8all[:], scalar=-8192.0, in1=c2048[:],
            op0=Alu.mult, op1=Alu.add)
        pad = outs.tile([128, 64], dt.float32, tag="pad")
        # pad = relu(u - 2047): 1 iff u == 2048 (unmatched), else 0
        nc.vector.scalar_tensor_tensor(
            out=pad[:], in0=u[:], scalar=-2047.0, in1=z64[:],
            op0=Alu.add, op1=Alu.max)
        # o = u - 2049*pad  -> j or -1 (int32 cast on write)
        o = outs.tile([128, 64], dt.int32, tag="o")
        nc.vector.scalar_tensor_tensor(
            out=o[:], in0=pad[:], scalar=-2049.0, in1=u[:],
            op0=Alu.mult, op1=Alu.add)
        for t in range(N_QT):
            nc.sync.dma_start(out_d[t * 128:(t + 1) * 128, 0:8],
                              o[:, 8 * t:8 * t + 8])
            nc.sync.dma_start(out_d[t * 128:(t + 1) * 128, 8:K_MAX], pad56[:])

    return nc


def _get_program():
    if "prog" not in _CACHE:
        nc = _build_program()
        if not nc.is_finalized():
            nc.finalize()  # Bacc: runs wait-splitting + reg-alloc passes
        _CACHE["prog"] = nc
    return _CACHE["prog"]


def _ramp_rows():
    """[2, L] bf16 rows summing (via the all-ones weight rows) to
    ramp(j) = (2048-j)*2^-13: hi = (128-(j>>4))*2^-9, lo = -(j&15)*2^-13.
    Every term is exactly representable in bf16, and relu(S'-32) lands in
    (0, 0.25] where fp16 spacing is <= 2^-13, so values stay exact."""
    import ml_dtypes
    j = np.arange(L)
    hi = (128 - (j >> 4)).astype(np.float32) * 2.0 ** -9
    lo = -(j & 15).astype(np.float32) * 2.0 ** -13
    return np.stack([hi, lo]).astype(ml_dtypes.bfloat16)


def _make_in_maps(q, k):
    ramp = _ramp_rows()
    in_maps = []
    for c in range(N_CORES):
        b, h = divmod(c, 2)
        qT = np.ascontiguousarray(q[b, h * QSH:(h + 1) * QSH, :].T)
        kT = np.ascontiguousarray(k[b].T)
        in_maps.append({"qT": qT, "kT": kT, "ramp": ramp})
    return in_maps


def run_device(q, k, trace=False):
    """Run the bass kernel on the 8 cores; returns (full_out, results_obj)."""
    from concourse.bass_utils import run_bass_kernel_spmd

    res = run_bass_kernel_spmd(
        _get_program(), _make_in_maps(q, k), list(range(N_CORES)), trace=trace)
    full = np.empty((B, L, K_MAX), np.int32)
    for c in range(N_CORES):
        b, h = divmod(c, 2)
        full[b, h * QSH:(h + 1) * QSH, :] = res.results[c]["out"]
    return full, res


def _reference_numpy(q, k):
    """Exact numpy fallback (used only if some row has >= 8 matches)."""
    out = np.full((B, L, K_MAX), -1, np.int32)
    for b in range(B):
        qb = (q[b] > 0)
        kb = (k[b] > 0)
        match = np.zeros((L, L), bool)
        for lo in (0, 32):
            qg = qb[:, lo:lo + 32]
            kg = kb[:, lo:lo + 32]
            # pack 32 bits into one uint32 per row for exact equality
            qc = np.packbits(qg, axis=1).view(">u4").ravel()
            kc = np.packbits(kg, axis=1).view(">u4").ravel()
            match |= qc[:, None] == kc[None, :]
        for i in range(L):
            idx = np.nonzero(match[i])[0][:K_MAX]
            out[b, i, :len(idx)] = idx
    return out


def kernel(query_up, key_up, head_idx=None, **_unused):
    q = np.asarray(query_up, dtype=np.float32)
    k = np.asarray(key_up, dtype=np.float32)
    assert q.shape == (B, L, D) and k.shape == (B, L, D)
    full, _ = run_device(q, k)
    # Exact overflow detection: a non(-1) 8th candidate means the row had
    # >= 8 matches, so candidates 9.. might have been dropped.
    if (full[..., 7] != -1).any():
        full = _reference_numpy(q, k)
    return full



# revision 10
# speedup vs baseline: 1.1064x; 1.1064x over previous
"""Trainium2 Bass kernel for the CandidateFinder sparse-attention problem.

Computes, for each (batch, query) row, the first K_MAX=64 key indices whose
32-bit sign pattern exactly matches the query's in either of two dim groups
(dims 0:32, 32:64), padded with -1.

Approach (per core; 8 cores = 4 batches x 2 query halves):
  - signs scaled to +-0.5 ((x>0) - 0.5 via one DVE tensor_scalar pass per
    group-tensor, exact, 4x mode on bf16 inputs); per group
    S_g[q,j]/4 = sum_d q_d k_d is a quarter-integer in [-8,8];
    match <=> S_g/4 == 8. (TensorE, K=34.)
  - two extra contraction rows add ramp(j) = (2048-j)*2^-13 (sum of two
    bf16-exact products), so P_g = S_g/4 + ramp is exact in fp32 PSUM and
    strictly decreasing in j for fixed S.
  - evacuation is balanced across ACT and DVE (DVE stt ops are 1x, plain
    tensor_tensor is 2x for fp16, so the merge ops are tt where possible):
      ACT evacuates three of the four [128,1024] PSUM blocks per query tile
      as relu(P - 8) -> fp16 (matched positions give exactly
      (2048-j)*2^-13, fp16-exact and descending in j; rest 0), summing the
      h0 blocks via the ACT accumulator;
      DVE evacuates the fourth fused with the h1 group merge
      (val_h1 = max(P_1 - 8, relu(P_2 - 8)), accum_out = sum) and merges
      the two ACT-evacuated h0 blocks with one all-fp16 tensor_tensor max.
  - two pairwise fp16 max folds shrink the row 2048 -> 512 before the DVE
    `max` (hardware top-8, descending) extracts the first <=8 matching j;
    max never alters values, so survivors still encode j exactly.
  - loss detection by sum conservation, batched per 4-tile half:
    upper = acc(v1h0) + acc(v2h0) + acc(val_h1) >= sum(val) with equality
    unless a key matches both groups (probability 2^-64, still flagged),
    and sum(top8) == sum(val) iff no fold collision dropped a match and
    the row had <= 8 matches. flag = upper > sum(top8) forces a positive
    8th slot, which triggers the exact host fallback.
  - three 2-source ops per half decode the top-8 values to j / -1 into a
    packed [128, 4*64] half of the output block; each half is written back
    with its own DMA so the first overlaps the second half's compute.
  - rows whose 8th candidate decodes as a real match (>=8 real matches, or
    the collision flag) are recomputed exactly on the host with numpy. With
    random normal inputs this never triggers: a match needs a 2^-32
    sign-pattern collision.

Self-contained: hardcodes shapes from the problem spec.
"""

import numpy as np

B = 4
L = 2048
D = 64
K_MAX = 64
N_CORES = 8
QSH = B * L // N_CORES  # 1024 queries per core
N_QT = QSH // 128       # 8 query tiles per core

_CACHE = {}


def _build_program(reps=1):
    from contextlib import ExitStack

    import concourse.bacc as bacc
    import concourse.mybir as mybir
    import concourse.tile as tile

    dt = mybir.dt
    Alu = mybir.AluOpType

    # Bacc (not raw Bass): its legalization passes split multi-sem waits,
    # which PE instructions can't carry (1 wait max per instruction).
    nc = bacc.Bacc("TRN2", target_bir_lowering=False, debug=False)
    # bf16 inputs: rounding fp32 -> bf16 preserves the sign bit, and bf16
    # flushes to 0.0 only below 1e-40, unreachable for randn fp32 data, so
    # (x > 0) is unchanged. Halves DMA bytes and lets the sign ops run in
    # the DVE's 4x perf mode.
    qT_d = nc.declare_dram_parameter("qT", [D, QSH], dt.bfloat16, isOutput=False)
    kT_d = nc.declare_dram_parameter("kT", [D, L], dt.bfloat16, isOutput=False)
    ramp_d = nc.declare_dram_parameter("ramp", [2, L], dt.bfloat16, isOutput=False)
    # packed output: out[p, 64*t + c] = candidate c of query row t*128 + p
    out_d = nc.declare_dram_parameter("out", [128, N_QT * K_MAX], dt.int32,
                                      isOutput=True)

    with tile.TileContext(nc) as tc, ExitStack() as ctx:
        consts = ctx.enter_context(tc.tile_pool(name="consts", bufs=1))
        vals = ctx.enter_context(tc.tile_pool(name="vals", bufs=3))
        outs = ctx.enter_context(tc.tile_pool(name="outs", bufs=2))
        psum = ctx.enter_context(tc.tile_pool(name="psum", bufs=1, space="PSUM"))

        # ---- load raw (transposed) inputs, split by dim group ----
        # qg1 from the ACT queue so its DGE setup parallels kg1's on SP;
        # the group-2 halves follow on SP.
        qraw = consts.tile([D, QSH], dt.bfloat16)
        kraw = consts.tile([D, L], dt.bfloat16)
        nc.scalar.dma_start(qraw[0:32, :], qT_d[0:32, :])
        nc.sync.dma_start(kraw[0:32, :], kT_d[0:32, :])
        nc.sync.dma_start(kraw[32:64, :], kT_d[32:64, :])
        nc.sync.dma_start(qraw[32:64, :], qT_d[32:64, :])

        # constants: all memsets on the otherwise-idle Pool engine, ordered
        # by when they are first needed
        bias8 = consts.tile([128, 1], dt.float32, tag="bias8")
        nc.gpsimd.memset(bias8[:], -8.0)
        c2048 = consts.tile([128, 64], dt.float32, tag="c2048")
        nc.gpsimd.memset(c2048[:], 2048.0)
        z64 = consts.tile([128, 64], dt.float32, tag="z64")
        nc.gpsimd.memset(z64[:], 0.0)
        # all 8 query tiles' top-8 values, decoded per 4-tile half
        t8all = consts.tile([128, 64], dt.float16, tag="t8all")
        # per-tile exact sums: cols 3t,3t+1 = ACT accums of the h0 blocks,
        # col 3t+2 = DVE accum of the merged h1 block
        svall = consts.tile([128, 3 * N_QT], dt.float32, tag="svall")
        # packed output block; -1 everywhere the decode doesn't overwrite
        o2 = consts.tile([128, N_QT * K_MAX], dt.int32, tag="o2")
        nc.gpsimd.memset(o2[:], -1)

        # ---- sign tiles (+ ramp rows) ----
        # QS[g]: [34, QSH]  rows 0:32 = query signs (+-0.5), rows 32/33 = 1.0
        # KS[g]: [34, L]    rows 0:32 = key signs (+-0.5), rows 32/33 = ramp
        # All four sign passes on DVE: bf16 in/out, SBUF, step 1 -> 4x mode.
        # (x>0) - 0.5 -> +-0.5 exactly; x == 0 -> -0.5 like the reference.
        QS = []
        KS = []
        for g in range(2):
            qs = consts.tile([34, QSH], dt.bfloat16, tag=f"qs{g}")
            ks = consts.tile([34, L], dt.bfloat16, tag=f"ks{g}")
            lo, hi = g * 32, (g + 1) * 32
            nc.vector.tensor_scalar(
                out=qs[0:32, :], in0=qraw[lo:hi, :],
                scalar1=0.0, scalar2=0.5, op0=Alu.is_gt, op1=Alu.subtract)
            nc.vector.tensor_scalar(
                out=ks[0:32, :], in0=kraw[lo:hi, :],
                scalar1=0.0, scalar2=0.5, op0=Alu.is_gt, op1=Alu.subtract)
            nc.gpsimd.memset(qs[32:34, :], 1.0)
            # ramp terms (host-precomputed bf16 constants) into rows 32/33
            nc.gpsimd.dma_start(ks[32:34, :], ramp_d[:])
            QS.append(qs)
            KS.append(ks)

        t8v = t8all.rearrange("p (t c) -> p t c", c=8)
        sv3 = svall.rearrange("p (t c) -> p t c", c=3)
        o2v = o2.rearrange("p (t c) -> p t c", c=K_MAX)

        def half_tail(h):
            """Flags + decode + writeback for tiles 4h..4h+3."""
            ts = slice(4 * h, 4 * h + 4)
            # upper bound on sum(val) from the three exact accums
            s01 = outs.tile([128, 4], dt.float32, tag="s01")
            nc.vector.tensor_tensor(
                out=s01[:], in0=sv3[:, ts, 0], in1=sv3[:, ts, 1], op=Alu.add)
            up4 = outs.tile([128, 4], dt.float32, tag="up4")
            nc.vector.tensor_tensor(
                out=up4[:], in0=s01[:], in1=sv3[:, ts, 2], op=Alu.add)
            # sum of the extracted top-8 values (exact in fp32)
            ts4 = outs.tile([128, 4], dt.float32, tag="ts4")
            nc.vector.tensor_reduce(
                out=ts4[:], in_=t8v[:, ts, :], axis=mybir.AxisListType.X,
                op=Alu.add)
            # flag = some match was dropped (fold collision, > 8 matches, or
            # a 2^-64 both-group match) -> force slot 7 positive
            fl4 = outs.tile([128, 4], dt.float32, tag="fl4")
            nc.vector.tensor_tensor(
                out=fl4[:], in0=up4[:], in1=ts4[:], op=Alu.is_gt)
            nc.vector.scalar_tensor_tensor(
                out=t8v[:, ts, 7], in0=fl4[:], scalar=2.0 ** -13,
                in1=t8v[:, ts, 7], op0=Alu.mult, op1=Alu.max)
            # decode: matched v = (2048-j)*2^-13 => u = 2048 - 8192*v = j;
            # unmatched v = 0 => u = 2048 -> -1.
            cols = slice(32 * h, 32 * h + 32)
            u = outs.tile([128, 32], dt.float32, tag="u")
            nc.vector.scalar_tensor_tensor(
                out=u[:], in0=t8all[:, cols], scalar=-8192.0,
                in1=c2048[:, 0:32], op0=Alu.mult, op1=Alu.add)
            pad = outs.tile([128, 32], dt.float32, tag="pad")
            nc.vector.scalar_tensor_tensor(
                out=pad[:], in0=u[:], scalar=-2047.0, in1=z64[:, 0:32],
                op0=Alu.add, op1=Alu.max)
            # o = u - 2049*pad -> j or -1 (int32 cast on write), scattered
            # into the first 8 columns of each tile's 64-column block
            nc.vector.scalar_tensor_tensor(
                out=o2v[:, ts, 0:8],
                in0=pad.rearrange("p (t c) -> p t c", c=8),
                scalar=-2049.0,
                in1=u.rearrange("p (t c) -> p t c", c=8),
                op0=Alu.mult, op1=Alu.add)
            nc.sync.dma_start(out_d[:, 256 * h:256 * (h + 1)],
                              o2[:, 256 * h:256 * (h + 1)])

        # ---- main loop over query tiles ----
        # reps>1 repeats the whole body inside one NEFF (timing only).
        for _ in range(reps):
            for t in range(N_QT):
                val = vals.tile([128, L], dt.float16, tag="val")
                v1h0 = vals.tile([128, 1024], dt.float16, tag="v1h0")
                v2h0 = vals.tile([128, 1024], dt.float16, tag="v2h0")
                v2h1 = vals.tile([128, 1024], dt.float16, tag="v2h1")

                # half 0: ACT evacuates both groups (with exact accums),
                # DVE merges with one all-fp16 2x tensor_tensor
                p0 = psum.tile([128, 1024], dt.float32, tag="p0")
                p1 = psum.tile([128, 1024], dt.float32, tag="p1")
                for g, pg in enumerate((p0, p1)):
                    for n in range(2):
                        nc.tensor.matmul(
                            pg[:, n * 512:(n + 1) * 512],
                            QS[g][:, t * 128:(t + 1) * 128],
                            KS[g][:, n * 512:(n + 1) * 512],
                            start=True, stop=True)
                nc.scalar.activation(
                    v1h0[:], p0[:], mybir.ActivationFunctionType.Relu,
                    bias=bias8[:], scale=1.0,
                    accum_out=sv3[:, t, 0:1])
                nc.scalar.activation(
                    v2h0[:], p1[:], mybir.ActivationFunctionType.Relu,
                    bias=bias8[:], scale=1.0,
                    accum_out=sv3[:, t, 1:2])
                nc.vector.tensor_tensor(
                    out=val[:, 0:1024], in0=v1h0[:], in1=v2h0[:], op=Alu.max)

                # half 1: ACT evacuates group 2, DVE evacuates group 1
                # fused with the merge (PSUM source, 1x) + exact accum
                p0b = psum.tile([128, 1024], dt.float32, tag="p0b")
                p1b = psum.tile([128, 1024], dt.float32, tag="p1b")
                for g, pg in enumerate((p0b, p1b)):
                    for n in range(2):
                        nc.tensor.matmul(
                            pg[:, n * 512:(n + 1) * 512],
                            QS[g][:, t * 128:(t + 1) * 128],
                            KS[g][:, 1024 + n * 512:1024 + (n + 1) * 512],
                            start=True, stop=True)
                nc.scalar.activation(
                    v2h1[:], p1b[:], mybir.ActivationFunctionType.Relu,
                    bias=bias8[:], scale=1.0)
                nc.vector.scalar_tensor_tensor(
                    out=val[:, 1024:2048], in0=p0b[:], scalar=-8.0,
                    in1=v2h1[:], op0=Alu.add, op1=Alu.max,
                    accum_out=sv3[:, t, 2:3])

                # fold 2048 -> 512 (all-fp16 2x tt) and extract the top-8
                m1 = vals.tile([128, L // 2], dt.float16, tag="m1")
                nc.vector.tensor_tensor(
                    out=m1[:], in0=val[:, 0:1024], in1=val[:, 1024:2048],
                    op=Alu.max)
                m2 = vals.tile([128, L // 4], dt.float16, tag="m2")
                nc.vector.tensor_tensor(
                    out=m2[:], in0=m1[:, 0:512], in1=m1[:, 512:1024],
                    op=Alu.max)
                nc.vector.max(t8all[:, 8 * t:8 * t + 8], m2[:])

                if t == 3:
                    half_tail(0)
            half_tail(1)

    return nc


def _get_program():
    if "prog" not in _CACHE:
        nc = _build_program()
        if not nc.is_finalized():
            nc.finalize()  # Bacc: runs wait-splitting + reg-alloc passes
        _CACHE["prog"] = nc
    return _CACHE["prog"]


def _ramp_rows():
    """[2, L] bf16 rows summing (via the all-ones weight rows) to
    ramp(j) = (2048-j)*2^-13: hi = (128-(j>>4))*2^-9, lo = -(j&15)*2^-13.
    Every term is exactly representable in bf16, and relu(P-16) lands in
    (0, 0.25] where fp16 spacing is <= 2^-13, so values stay exact."""
    import ml_dtypes
    j = np.arange(L)
    hi = (128 - (j >> 4)).astype(np.float32) * 2.0 ** -9
    lo = -(j & 15).astype(np.float32) * 2.0 ** -13
    return np.stack([hi, lo]).astype(ml_dtypes.bfloat16)


def _make_in_maps(q, k):
    import ml_dtypes
    ramp = _ramp_rows()
    in_maps = []
    for c in range(N_CORES):
        b, h = divmod(c, 2)
        # bf16 rounding preserves (x > 0) for all reachable randn fp32
        qT = np.ascontiguousarray(
            q[b, h * QSH:(h + 1) * QSH, :].T.astype(ml_dtypes.bfloat16))
        kT = np.ascontiguousarray(k[b].T.astype(ml_dtypes.bfloat16))
        in_maps.append({"qT": qT, "kT": kT, "ramp": ramp})
    return in_maps


def run_device(q, k, trace=False):
    """Run the bass kernel on the 8 cores; returns (full_out, results_obj)."""
    from concourse.bass_utils import run_bass_kernel_spmd

    res = run_bass_kernel_spmd(
        _get_program(), _make_in_maps(q, k), list(range(N_CORES)), trace=trace)
    full = np.empty((B, L, K_MAX), np.int32)
    for c in range(N_CORES):
        b, h = divmod(c, 2)
        # out[p, 64t+c] = result for query row t*128+p
        blk = res.results[c]["out"].reshape(128, N_QT, K_MAX)
        full[b, h * QSH:(h + 1) * QSH, :] = (
            blk.transpose(1, 0, 2).reshape(QSH, K_MAX))
    return full, res


def _reference_numpy(q, k):
    """Exact numpy fallback (used only if some row has >= 8 matches)."""
    out = np.full((B, L, K_MAX), -1, np.int32)
    for b in range(B):
        qb = (q[b] > 0)
        kb = (k[b] > 0)
        match = np.zeros((L, L), bool)
        for lo in (0, 32):
            qg = qb[:, lo:lo + 32]
            kg = kb[:, lo:lo + 32]
            # pack 32 bits into one uint32 per row for exact equality
            qc = np.packbits(qg, axis=1).view(">u4").ravel()
            kc = np.packbits(kg, axis=1).view(">u4").ravel()
            match |= qc[:, None] == kc[None, :]
        for i in range(L):
            idx = np.nonzero(match[i])[0][:K_MAX]
            out[b, i, :len(idx)] = idx
    return out


def kernel(query_up, key_up, head_idx=None, **_unused):
    q = np.asarray(query_up, dtype=np.float32)
    k = np.asarray(key_up, dtype=np.float32)
    assert q.shape == (B, L, D) and k.shape == (B, L, D)
    full, _ = run_device(q, k)
    # Exact overflow detection: a non(-1) 8th candidate means the row had
    # >= 8 matches, so candidates 9.. might have been dropped.
    if (full[..., 7] != -1).any():
        full = _reference_numpy(q, k)
    return full


# revision 39
# speedup vs baseline: 1.2225x; 1.1049x over previous
"""Trainium2 Bass kernel for the CandidateFinder sparse-attention problem.

Computes, for each (batch, query) row, the first K_MAX=64 key indices whose
32-bit sign pattern exactly matches the query's in either of two dim groups
(dims 0:32, 32:64), padded with -1.

Approach (per core; 8 cores = 4 batches x 2 query halves):
  - signs scaled to +-0.5 ((x>0) - 0.5 via one DVE tensor_scalar pass per
    group-tensor, exact, 4x mode on bf16 inputs); per group
    S_g[q,j]/4 = sum_d q_d k_d is a quarter-integer in [-8,8];
    match <=> S_g/4 == 8. (TensorE, K=34.)
  - two extra contraction rows add ramp(j) = (2048-j)*2^-13 (sum of two
    bf16-exact products), so P_g = S_g/4 + ramp is exact in fp32 PSUM and
    strictly decreasing in j for fixed S.
  - evacuation is balanced across ACT and DVE (DVE stt ops are 1x, plain
    tensor_tensor is 2x for fp16, so the merge ops are tt where possible):
      ACT evacuates three of the four [128,1024] PSUM blocks per query tile
      as relu(P - 8) -> fp16 (matched positions give exactly
      (2048-j)*2^-13, fp16-exact and descending in j; rest 0), summing the
      h0 blocks via the ACT accumulator;
      DVE evacuates the fourth fused with the h1 group merge
      (val_h1 = max(P_1 - 8, relu(P_2 - 8)), accum_out = sum) and merges
      the two ACT-evacuated h0 blocks with one all-fp16 tensor_tensor max.
  - two pairwise fp16 max folds shrink the row 2048 -> 512 before the DVE
    `max` (hardware top-8, descending) extracts the first <=8 matching j;
    max never alters values, so survivors still encode j exactly.
  - loss detection by sum conservation, batched per 4-tile half:
    upper = acc(v1h0) + acc(v2h0) + acc(val_h1) >= sum(val) with equality
    unless a key matches both groups (probability 2^-64, still flagged),
    and sum(top8) == sum(val) iff no fold collision dropped a match and
    the row had <= 8 matches. flag = upper > sum(top8) forces a positive
    8th slot, which triggers the exact host fallback.
  - three 2-source ops per half decode the top-8 values to j / -1 into a
    packed [128, 4*64] half of the output block; each half is written back
    with its own DMA so the first overlaps the second half's compute.
  - rows whose 8th candidate decodes as a real match (>=8 real matches, or
    the collision flag) are recomputed exactly on the host with numpy. With
    random normal inputs this never triggers: a match needs a 2^-32
    sign-pattern collision.

Self-contained: hardcodes shapes from the problem spec.
"""

import numpy as np

B = 4
L = 2048
D = 64
K_MAX = 64
N_CORES = 8
QSH = B * L // N_CORES  # 1024 queries per core
N_QT = QSH // 128       # 8 query tiles per core

_CACHE = {}


def _build_program(reps=1):
    from contextlib import ExitStack

    import concourse.bacc as bacc
    import concourse.mybir as mybir
    import concourse.tile as tile

    dt = mybir.dt
    Alu = mybir.AluOpType

    # Bacc (not raw Bass): its legalization passes split multi-sem waits,
    # which PE instructions can't carry (1 wait max per instruction).
    nc = bacc.Bacc("TRN2", target_bir_lowering=False, debug=False)
    # bf16 inputs: rounding fp32 -> bf16 preserves the sign bit, and bf16
    # flushes to 0.0 only below 1e-40, unreachable for randn fp32 data, so
    # (x > 0) is unchanged. Halves DMA bytes and lets the sign ops run in
    # the DVE's 4x perf mode.
    qT_d = nc.declare_dram_parameter("qT", [D, QSH], dt.bfloat16, isOutput=False)
    kT_d = nc.declare_dram_parameter("kT", [D, L], dt.bfloat16, isOutput=False)
    ramp_d = nc.declare_dram_parameter("ramp", [2, L], dt.bfloat16, isOutput=False)
    # packed output: out[p, 64*t + c] = candidate c of query row t*128 + p
    out_d = nc.declare_dram_parameter("out", [128, N_QT * K_MAX], dt.int32,
                                      isOutput=True)

    with tile.TileContext(nc) as tc, ExitStack() as ctx:
        consts = ctx.enter_context(tc.tile_pool(name="consts", bufs=1))
        vals = ctx.enter_context(tc.tile_pool(name="vals", bufs=3))
        outs = ctx.enter_context(tc.tile_pool(name="outs", bufs=2))
        psum = ctx.enter_context(tc.tile_pool(name="psum", bufs=1, space="PSUM"))

        # ---- load raw (transposed) inputs, split by dim group ----
        # all on SP (the ACT queue would stall a DMA setup behind the
        # activation table load); group 1 first — it gates the first signs
        qraw = consts.tile([D, QSH], dt.bfloat16)
        kraw = consts.tile([D, L], dt.bfloat16)
        nc.sync.dma_start(kraw[0:32, :], kT_d[0:32, :])
        nc.sync.dma_start(qraw[0:32, :], qT_d[0:32, :])
        nc.sync.dma_start(kraw[32:64, :], kT_d[32:64, :])
        nc.sync.dma_start(qraw[32:64, :], qT_d[32:64, :])

        # constants: all memsets/small DMAs on the otherwise-idle Pool
        # engine, ordered by when they are first needed (z64 first: it
        # feeds the PE warmup; the ramp DMAs gate the first matmuls so
        # they precede the slower ones-row memsets)
        z64 = consts.tile([128, 64], dt.float32, tag="z64")
        nc.gpsimd.memset(z64[:], 0.0)
        bias8 = consts.tile([128, 1], dt.float32, tag="bias8")
        nc.gpsimd.memset(bias8[:], -8.0)

        # ACT warmup: a dependency-free activation so the ~1.3us Relu
        # table load happens at t~0 instead of attaching itself (and its
        # successor's waits) to the first real evacuation.
        actw = consts.tile([128, 1], dt.float32, tag="actw")
        nc.scalar.activation(
            actw[:], z64[:, 0:1], mybir.ActivationFunctionType.Relu,
            bias=0.0, scale=1.0)

        # ---- sign tiles (+ ramp rows) ----
        # QS[g]: [34, QSH]  rows 0:32 = query signs (+-0.5), rows 32/33 = 1.0
        # KS[g]: [34, L]    rows 0:32 = key signs (+-0.5), rows 32/33 = ramp
        # All four sign passes on DVE: bf16 in/out, SBUF, step 1 -> 4x mode.
        # (x>0) - 0.5 -> +-0.5 exactly; x == 0 -> -0.5 like the reference.
        QS = []
        KS = []
        for g in range(2):
            qs = consts.tile([34, QSH], dt.bfloat16, tag=f"qs{g}")
            ks = consts.tile([34, L], dt.bfloat16, tag=f"ks{g}")
            QS.append(qs)
            KS.append(ks)
            # ramp terms (host-precomputed bf16 constants) into rows 32/33
            nc.gpsimd.dma_start(ks[32:34, :], ramp_d[:])
        for g in range(2):
            nc.gpsimd.memset(QS[g][32:34, :], 1.0)
        # key signs first: they are the longer pass and gate the first
        # matmul together with the query signs
        for g in range(2):
            lo, hi = g * 32, (g + 1) * 32
            nc.vector.tensor_scalar(
                out=KS[g][0:32, :], in0=kraw[lo:hi, :],
                scalar1=0.0, scalar2=0.5, op0=Alu.is_gt, op1=Alu.subtract)
            nc.vector.tensor_scalar(
                out=QS[g][0:32, :], in0=qraw[lo:hi, :],
                scalar1=0.0, scalar2=0.5, op0=Alu.is_gt, op1=Alu.subtract)

        c2048 = consts.tile([128, 64], dt.float32, tag="c2048")
        nc.gpsimd.memset(c2048[:], 2048.0)
        # all 8 query tiles' top-8 values, decoded per 4-tile half
        t8all = consts.tile([128, 64], dt.float16, tag="t8all")
        # per-tile exact sums: col 2t = Pool accum of the merged h0 block,
        # col 2t+1 = DVE accum of the merged h1 block (both exact: a
        # both-group match contributes once, like the reference's union)
        svall = consts.tile([128, 2 * N_QT], dt.float32, tag="svall")
        # packed output block; -1 everywhere the decode doesn't overwrite
        o2 = consts.tile([128, N_QT * K_MAX], dt.int32, tag="o2")
        nc.gpsimd.memset(o2[:], -1)

        # ---- PE warmup: ~10 zero matmuls so the HAM clock gate reaches
        # full speed before the first real matmuls (the activity window is
        # ~3.4us; these run while the DMAs and signs are still in flight).
        # z64 is all zeros so the scratch PSUM results are finite; they
        # reuse the p01 storage, which tile 0 overwrites with start=True.
        pwarm = psum.tile([128, 1024], dt.float32, tag="p0b")
        for w in range(10):
            nc.tensor.matmul(
                pwarm[0:64, w * 64:w * 64 + 64],
                z64[:], z64[:, 0:64], start=True, stop=True)

        t8v = t8all.rearrange("p (t c) -> p t c", c=8)
        sv2 = svall.rearrange("p (t c) -> p t c", c=2)
        o2v = o2.rearrange("p (t c) -> p t c", c=K_MAX)

        def half_tail(h):
            """Flags + decode + writeback for tiles 4h..4h+3."""
            ts = slice(4 * h, 4 * h + 4)
            # sum(val) from the two exact merged-block accums
            up4 = outs.tile([128, 4], dt.float32, tag="up4")
            nc.vector.tensor_tensor(
                out=up4[:], in0=sv2[:, ts, 0], in1=sv2[:, ts, 1], op=Alu.add)
            # sum of the extracted top-8 values (exact in fp32)
            ts4 = outs.tile([128, 4], dt.float32, tag="ts4")
            nc.vector.tensor_reduce(
                out=ts4[:], in_=t8v[:, ts, :], axis=mybir.AxisListType.X,
                op=Alu.add)
            # flag = some match was dropped (fold collision, > 8 matches, or
            # a 2^-64 both-group match) -> force slot 7 positive
            fl4 = outs.tile([128, 4], dt.float32, tag="fl4")
            nc.vector.tensor_tensor(
                out=fl4[:], in0=up4[:], in1=ts4[:], op=Alu.is_gt)
            nc.vector.scalar_tensor_tensor(
                out=t8v[:, ts, 7], in0=fl4[:], scalar=2.0 ** -13,
                in1=t8v[:, ts, 7], op0=Alu.mult, op1=Alu.max)
            # decode: matched v = (2048-j)*2^-13 => u = 2048 - 8192*v = j;
            # unmatched v = 0 => u = 2048 -> -1.
            cols = slice(32 * h, 32 * h + 32)
            u = outs.tile([128, 32], dt.float32, tag="u")
            nc.vector.scalar_tensor_tensor(
                out=u[:], in0=t8all[:, cols], scalar=-8192.0,
                in1=c2048[:, 0:32], op0=Alu.mult, op1=Alu.add)
            pad = outs.tile([128, 32], dt.float32, tag="pad")
            nc.vector.scalar_tensor_tensor(
                out=pad[:], in0=u[:], scalar=-2047.0, in1=z64[:, 0:32],
                op0=Alu.add, op1=Alu.max)
            # o = u - 2049*pad -> j or -1 (int32 cast on write), scattered
            # into the first 8 columns of each tile's 64-column block
            nc.vector.scalar_tensor_tensor(
                out=o2v[:, ts, 0:8],
                in0=pad.rearrange("p (t c) -> p t c", c=8),
                scalar=-2049.0,
                in1=u.rearrange("p (t c) -> p t c", c=8),
                op0=Alu.mult, op1=Alu.add)
            # writeback from SP (idle by now; cheaper DGE setup than SWDGE)
            nc.sync.dma_start(out_d[:, 256 * h:256 * (h + 1)],
                              o2[:, 256 * h:256 * (h + 1)])

        # ---- main loop over query tiles ----
        # reps>1 repeats the whole body inside one NEFF (timing only).
        for _ in range(reps):
            for t in range(N_QT):
                val = vals.tile([128, 1024], dt.float16, tag="val")
                vh0 = vals.tile([128, 2048], dt.float16, tag="vh0")
                v2h1 = vals.tile([128, 1024], dt.float16, tag="v2h1")

                # half 0: both groups' matmuls land in one 4-bank PSUM tile
                # so ONE 2048-wide ACT relu evacuates both; its accumulator
                # gives sum(vh0) >= sum(val_h0), tight unless a 2^-64
                # both-group match (which then just false-positives the
                # exact host fallback). DVE merges with one all-fp16 2x tt.
                p01 = psum.tile([128, 2048], dt.float32, tag="p01")
                for g in range(2):
                    for n in range(2):
                        nc.tensor.matmul(
                            p01[:, g * 1024 + n * 512:g * 1024 + (n + 1) * 512],
                            QS[g][:, t * 128:(t + 1) * 128],
                            KS[g][:, n * 512:(n + 1) * 512],
                            start=True, stop=True)
                nc.scalar.activation(
                    vh0[:], p01[:], mybir.ActivationFunctionType.Relu,
                    bias=bias8[:], scale=1.0,
                    accum_out=sv2[:, t, 0:1])
                nc.vector.tensor_tensor(
                    out=val[:, 0:1024], in0=vh0[:, 0:1024],
                    in1=vh0[:, 1024:2048], op=Alu.max)

                # half 1: ACT evacuates group 2 (own PSUM tile), DVE
                # evacuates group 1 fused with the merge (PSUM source, 1x)
                # + exact accum; separate tiles keep the two loops
                # independent.
                p0b = psum.tile([128, 1024], dt.float32, tag="p0b")
                p1b = psum.tile([128, 1024], dt.float32, tag="p1b")
                for g, pg in ((1, p1b), (0, p0b)):
                    for n in range(2):
                        nc.tensor.matmul(
                            pg[:, n * 512:(n + 1) * 512],
                            QS[g][:, t * 128:(t + 1) * 128],
                            KS[g][:, 1024 + n * 512:1024 + (n + 1) * 512],
                            start=True, stop=True)
                nc.scalar.activation(
                    v2h1[:], p1b[:], mybir.ActivationFunctionType.Relu,
                    bias=bias8[:], scale=1.0)
                val1 = vals.tile([128, 1024], dt.float16, tag="val1")
                nc.vector.scalar_tensor_tensor(
                    out=val1[:], in0=p0b[:], scalar=-8.0,
                    in1=v2h1[:], op0=Alu.add, op1=Alu.max,
                    accum_out=sv2[:, t, 1:2])

                # fold 2048 -> 256 (all-fp16 2x tt) and extract the top-8
                m1 = vals.tile([128, 1024], dt.float16, tag="m1")
                nc.vector.tensor_tensor(
                    out=m1[:], in0=val[:, 0:1024], in1=val1[:], op=Alu.max)
                m2 = vals.tile([128, 512], dt.float16, tag="m2")
                nc.vector.tensor_tensor(
                    out=m2[:], in0=m1[:, 0:512], in1=m1[:, 512:1024],
                    op=Alu.max)
                m3 = vals.tile([128, 256], dt.float16, tag="m3")
                nc.vector.tensor_tensor(
                    out=m3[:], in0=m2[:, 0:256], in1=m2[:, 256:512],
                    op=Alu.max)
                nc.vector.max(t8all[:, 8 * t:8 * t + 8], m3[:])

                if t == 3:
                    half_tail(0)
            half_tail(1)

    return nc


def _get_program():
    if "prog" not in _CACHE:
        nc = _build_program()
        if not nc.is_finalized():
            nc.finalize()  # Bacc: runs wait-splitting + reg-alloc passes
        _CACHE["prog"] = nc
    return _CACHE["prog"]


def _ramp_rows():
    """[2, L] bf16 rows summing (via the all-ones weight rows) to
    ramp(j) = (2048-j)*2^-13: hi = (128-(j>>4))*2^-9, lo = -(j&15)*2^-13.
    Every term is exactly representable in bf16, and relu(P-16) lands in
    (0, 0.25] where fp16 spacing is <= 2^-13, so values stay exact."""
    import ml_dtypes
    j = np.arange(L)
    hi = (128 - (j >> 4)).astype(np.float32) * 2.0 ** -9
    lo = -(j & 15).astype(np.float32) * 2.0 ** -13
    return np.stack([hi, lo]).astype(ml_dtypes.bfloat16)


def _make_in_maps(q, k):
    import ml_dtypes
    ramp = _ramp_rows()
    in_maps = []
    for c in range(N_CORES):
        b, h = divmod(c, 2)
        # bf16 rounding preserves (x > 0) for all reachable randn fp32
        qT = np.ascontiguousarray(
            q[b, h * QSH:(h + 1) * QSH, :].T.astype(ml_dtypes.bfloat16))
        kT = np.ascontiguousarray(k[b].T.astype(ml_dtypes.bfloat16))
        in_maps.append({"qT": qT, "kT": kT, "ramp": ramp})
    return in_maps


def run_device(q, k, trace=False):
    """Run the bass kernel on the 8 cores; returns (full_out, results_obj)."""
    from concourse.bass_utils import run_bass_kernel_spmd

    res = run_bass_kernel_spmd(
        _get_program(), _make_in_maps(q, k), list(range(N_CORES)), trace=trace)
    full = np.empty((B, L, K_MAX), np.int32)
    for c in range(N_CORES):
        b, h = divmod(c, 2)
        # out[p, 64t+c] = result for query row t*128+p
        blk = res.results[c]["out"].reshape(128, N_QT, K_MAX)
        full[b, h * QSH:(h + 1) * QSH, :] = (
            blk.transpose(1, 0, 2).reshape(QSH, K_MAX))
    return full, res


def _reference_numpy(q, k):
    """Exact numpy fallback (used only if some row has >= 8 matches)."""
    out = np.full((B, L, K_MAX), -1, np.int32)
    for b in range(B):
        qb = (q[b] > 0)
        kb = (k[b] > 0)
        match = np.zeros((L, L), bool)
        for lo in (0, 32):
            qg = qb[:, lo:lo + 32]
            kg = kb[:, lo:lo + 32]
            # pack 32 bits into one uint32 per row for exact equality
            qc = np.packbits(qg, axis=1).view(">u4").ravel()
            kc = np.packbits(kg, axis=1).view(">u4").ravel()
            match |= qc[:, None] == kc[None, :]
        for i in range(L):
            idx = np.nonzero(match[i])[0][:K_MAX]
            out[b, i, :len(idx)] = idx
    return out


def kernel(query_up, key_up, head_idx=None, **_unused):
    q = np.asarray(query_up, dtype=np.float32)
    k = np.asarray(key_up, dtype=np.float32)
    assert q.shape == (B, L, D) and k.shape == (B, L, D)
    full, _ = run_device(q, k)
    # Exact overflow detection: a non(-1) 8th candidate means the row had
    # >= 8 matches, so candidates 9.. might have been dropped.
    if (full[..., 7] != -1).any():
        full = _reference_numpy(q, k)
    return full


# revision 43
# speedup vs baseline: 1.3060x; 1.0683x over previous
"""Trainium2 Bass kernel for the CandidateFinder sparse-attention problem.

Computes, for each (batch, query) row, the first K_MAX=64 key indices whose
32-bit sign pattern exactly matches the query's in either of two dim groups
(dims 0:32, 32:64), padded with -1.

Approach (per core; 8 cores = 4 batches x 2 query halves):
  - inputs arrive as bf16 (sign-preserving host cast, halves DMA bytes);
    signs scaled to +-0.5 ((x>0) - 0.5 via one DVE tensor_scalar pass per
    group-tensor, exact, 4x mode on bf16); per group
    S_g[q,j]/4 = sum_d q_d k_d is a quarter-integer in [-8,8];
    match <=> S_g/4 == 8. (TensorE, K=34.)
  - two extra contraction rows add ramp(j) = (2048-j)*2^-13 (sum of two
    bf16-exact products), so P_g = S_g/4 + ramp is exact in fp32 PSUM and
    strictly decreasing in j for fixed S.
  - per query tile, the key axis is processed in two PSUM phases:
      h0 (keys 0:1024): both groups' matmuls land in one 4-bank PSUM tile;
      ONE 2048-wide ACT relu(P - 8) evacuates it to fp16 (matched
      positions give exactly (2048-j)*2^-13, rest 0), and its accumulator
      gives an exact-unless-2^-64 upper bound on the merged sum; a DVE
      all-fp16 tensor_tensor max (2x mode) merges the two groups.
      h1 (keys 1024:2048): the groups use separate PSUM tiles so their
      evacuations free them independently (PSUM dependencies are tracked
      per tile): ACT relus group 2 while DVE evacuates group 1 fused with
      the merge (scalar_tensor_tensor, accum_out = exact merged sum).
    ACT and DVE run near-balanced (~3.1us each per tile); PE warmup
    matmuls at t~0 lift the HAM clock gate, and a dependency-free ACT
    warmup pulls the ~1.3us activation-table load off the critical path.
  - three pairwise fp16 max folds shrink the row 2048 -> 256 before the
    DVE `max` (hardware top-8, descending) extracts the first <=8
    matching j; max never alters values, so survivors encode j exactly.
  - loss detection by sum conservation, batched per 4-tile half:
    sum(top8) == acc(h0) + acc(h1) iff no fold collision dropped a match
    and the row had <= 8 matches; flag = (sum > top8sum) forces a
    positive 8th slot, which triggers the exact host fallback.
  - three 2-source ops per half decode the top-8 values to j / -1 into a
    packed [128, 4*64] half of the output block; each half is written back
    with its own DMA so the first overlaps the second half's compute.
  - rows whose 8th candidate decodes as a real match (>=8 real matches, or
    the collision flag) are recomputed exactly on the host with numpy. With
    random normal inputs this never triggers: a match needs a 2^-32
    sign-pattern collision.

Self-contained: hardcodes shapes from the problem spec.
"""

import numpy as np

B = 4
L = 2048
D = 64
K_MAX = 64
N_CORES = 8
QSH = B * L // N_CORES  # 1024 queries per core
N_QT = QSH // 128       # 8 query tiles per core

_CACHE = {}


def _build_program(reps=1):
    from contextlib import ExitStack

    import concourse.bacc as bacc
    import concourse.mybir as mybir
    import concourse.tile as tile

    dt = mybir.dt
    Alu = mybir.AluOpType

    # Bacc (not raw Bass): its legalization passes split multi-sem waits,
    # which PE instructions can't carry (1 wait max per instruction).
    nc = bacc.Bacc("TRN2", target_bir_lowering=False, debug=False)
    # bf16 inputs: rounding fp32 -> bf16 preserves the sign bit, and bf16
    # flushes to 0.0 only below 1e-40, unreachable for randn fp32 data, so
    # (x > 0) is unchanged. Halves DMA bytes and lets the sign ops run in
    # the DVE's 4x perf mode.
    qT_d = nc.declare_dram_parameter("qT", [D, QSH], dt.bfloat16, isOutput=False)
    kT_d = nc.declare_dram_parameter("kT", [D, L], dt.bfloat16, isOutput=False)
    ramp_d = nc.declare_dram_parameter("ramp", [2, L], dt.bfloat16, isOutput=False)
    # packed output: out[p, 64*t + c] = candidate c of query row t*128 + p
    out_d = nc.declare_dram_parameter("out", [128, N_QT * K_MAX], dt.int32,
                                      isOutput=True)

    with tile.TileContext(nc) as tc, ExitStack() as ctx:
        consts = ctx.enter_context(tc.tile_pool(name="consts", bufs=1))
        vals = ctx.enter_context(tc.tile_pool(name="vals", bufs=3))
        outs = ctx.enter_context(tc.tile_pool(name="outs", bufs=2))
        psum = ctx.enter_context(tc.tile_pool(name="psum", bufs=1, space="PSUM"))

        # ---- load raw (transposed) inputs, split by dim group ----
        # all on SP (the ACT queue would stall a DMA setup behind the
        # activation table load); group 1 first — it gates the first signs
        qraw = consts.tile([D, QSH], dt.bfloat16)
        kraw = consts.tile([D, L], dt.bfloat16)
        nc.sync.dma_start(kraw[0:32, :], kT_d[0:32, :])
        nc.sync.dma_start(qraw[0:32, :], qT_d[0:32, :])
        nc.sync.dma_start(kraw[32:64, :], kT_d[32:64, :])
        nc.sync.dma_start(qraw[32:64, :], qT_d[32:64, :])

        # constants: all memsets/small DMAs on the otherwise-idle Pool
        # engine, ordered by when they are first needed (z64 first: it
        # feeds the PE warmup; the ramp DMAs gate the first matmuls so
        # they precede the slower ones-row memsets)
        z64 = consts.tile([128, 64], dt.float32, tag="z64")
        nc.gpsimd.memset(z64[:], 0.0)
        bias8 = consts.tile([128, 1], dt.float32, tag="bias8")
        nc.gpsimd.memset(bias8[:], -8.0)

        # ACT warmup: a dependency-free activation so the ~1.3us Relu
        # table load happens at t~0 instead of attaching itself (and its
        # successor's waits) to the first real evacuation.
        actw = consts.tile([128, 1], dt.float32, tag="actw")
        nc.scalar.activation(
            actw[:], z64[:, 0:1], mybir.ActivationFunctionType.Relu,
            bias=0.0, scale=1.0)

        # ---- sign tiles (+ ramp rows) ----
        # QS[g]: [34, QSH]  rows 0:32 = query signs (+-0.5), rows 32/33 = 1.0
        # KS[g]: [34, L]    rows 0:32 = key signs (+-0.5), rows 32/33 = ramp
        # All four sign passes on DVE: bf16 in/out, SBUF, step 1 -> 4x mode.
        # (x>0) - 0.5 -> +-0.5 exactly; x == 0 -> -0.5 like the reference.
        QS = []
        KS = []
        for g in range(2):
            qs = consts.tile([34, QSH], dt.bfloat16, tag=f"qs{g}")
            ks = consts.tile([34, L], dt.bfloat16, tag=f"ks{g}")
            QS.append(qs)
            KS.append(ks)
            # ramp terms (host-precomputed bf16 constants) into rows 32/33
            nc.gpsimd.dma_start(ks[32:34, :], ramp_d[:])
        for g in range(2):
            nc.gpsimd.memset(QS[g][32:34, :], 1.0)
        # key signs first: they are the longer pass and gate the first
        # matmul together with the query signs
        for g in range(2):
            lo, hi = g * 32, (g + 1) * 32
            nc.vector.tensor_scalar(
                out=KS[g][0:32, :], in0=kraw[lo:hi, :],
                scalar1=0.0, scalar2=0.5, op0=Alu.is_gt, op1=Alu.subtract)
            nc.vector.tensor_scalar(
                out=QS[g][0:32, :], in0=qraw[lo:hi, :],
                scalar1=0.0, scalar2=0.5, op0=Alu.is_gt, op1=Alu.subtract)

        c2048 = consts.tile([128, 64], dt.float32, tag="c2048")
        nc.gpsimd.memset(c2048[:], 2048.0)
        # all 8 query tiles' top-8 values, decoded per 4-tile half
        t8all = consts.tile([128, 64], dt.float16, tag="t8all")
        # per-tile exact sums: col 2t = Pool accum of the merged h0 block,
        # col 2t+1 = DVE accum of the merged h1 block (both exact: a
        # both-group match contributes once, like the reference's union)
        svall = consts.tile([128, 2 * N_QT], dt.float32, tag="svall")
        # packed output block; -1 everywhere the decode doesn't overwrite
        o2 = consts.tile([128, N_QT * K_MAX], dt.int32, tag="o2")
        nc.gpsimd.memset(o2[:], -1)

        # ---- PE warmup: ~10 zero matmuls so the HAM clock gate reaches
        # full speed before the first real matmuls (the activity window is
        # ~3.4us; these run while the DMAs and signs are still in flight).
        # z64 is all zeros so the scratch PSUM results are finite; they
        # reuse the p01 storage, which tile 0 overwrites with start=True.
        pwarm = psum.tile([128, 1024], dt.float32, tag="p0b")
        for w in range(10):
            nc.tensor.matmul(
                pwarm[0:64, w * 64:w * 64 + 64],
                z64[:], z64[:, 0:64], start=True, stop=True)

        t8v = t8all.rearrange("p (t c) -> p t c", c=8)
        sv2 = svall.rearrange("p (t c) -> p t c", c=2)
        o2v = o2.rearrange("p (t c) -> p t c", c=K_MAX)

        def half_tail(h):
            """Flags + decode + writeback for tiles 4h..4h+3."""
            ts = slice(4 * h, 4 * h + 4)
            # sum(val) from the two exact merged-block accums
            up4 = outs.tile([128, 4], dt.float32, tag="up4")
            nc.vector.tensor_tensor(
                out=up4[:], in0=sv2[:, ts, 0], in1=sv2[:, ts, 1], op=Alu.add)
            # sum of the extracted top-8 values (exact in fp32)
            ts4 = outs.tile([128, 4], dt.float32, tag="ts4")
            nc.vector.tensor_reduce(
                out=ts4[:], in_=t8v[:, ts, :], axis=mybir.AxisListType.X,
                op=Alu.add)
            # flag = some match was dropped (fold collision, > 8 matches, or
            # a 2^-64 both-group match) -> force slot 7 positive
            fl4 = outs.tile([128, 4], dt.float32, tag="fl4")
            nc.vector.tensor_tensor(
                out=fl4[:], in0=up4[:], in1=ts4[:], op=Alu.is_gt)
            nc.vector.scalar_tensor_tensor(
                out=t8v[:, ts, 7], in0=fl4[:], scalar=2.0 ** -13,
                in1=t8v[:, ts, 7], op0=Alu.mult, op1=Alu.max)
            # decode: matched v = (2048-j)*2^-13 => u = 2048 - 8192*v = j;
            # unmatched v = 0 => u = 2048 -> -1.
            cols = slice(32 * h, 32 * h + 32)
            u = outs.tile([128, 32], dt.float32, tag="u")
            nc.vector.scalar_tensor_tensor(
                out=u[:], in0=t8all[:, cols], scalar=-8192.0,
                in1=c2048[:, 0:32], op0=Alu.mult, op1=Alu.add)
            pad = outs.tile([128, 32], dt.float32, tag="pad")
            nc.vector.scalar_tensor_tensor(
                out=pad[:], in0=u[:], scalar=-2047.0, in1=z64[:, 0:32],
                op0=Alu.add, op1=Alu.max)
            # o = u - 2049*pad -> j or -1 (int32 cast on write), scattered
            # into the first 8 columns of each tile's 64-column block
            nc.vector.scalar_tensor_tensor(
                out=o2v[:, ts, 0:8],
                in0=pad.rearrange("p (t c) -> p t c", c=8),
                scalar=-2049.0,
                in1=u.rearrange("p (t c) -> p t c", c=8),
                op0=Alu.mult, op1=Alu.add)
            # writeback from SP (idle by now; cheaper DGE setup than SWDGE)
            nc.sync.dma_start(out_d[:, 256 * h:256 * (h + 1)],
                              o2[:, 256 * h:256 * (h + 1)])

        # ---- main loop over query tiles ----
        # reps>1 repeats the whole body inside one NEFF (timing only).
        for _ in range(reps):
            for t in range(N_QT):
                val = vals.tile([128, 1024], dt.float16, tag="val")
                vh0 = vals.tile([128, 2048], dt.float16, tag="vh0")
                v2h1 = vals.tile([128, 1024], dt.float16, tag="v2h1")

                # half 0: both groups' matmuls land in one 4-bank PSUM tile
                # so ONE 2048-wide ACT relu evacuates both; its accumulator
                # gives sum(vh0) >= sum(val_h0), tight unless a 2^-64
                # both-group match (which then just false-positives the
                # exact host fallback). DVE merges with one all-fp16 2x tt.
                p01 = psum.tile([128, 2048], dt.float32, tag="p01")
                for g in range(2):
                    for n in range(2):
                        nc.tensor.matmul(
                            p01[:, g * 1024 + n * 512:g * 1024 + (n + 1) * 512],
                            QS[g][:, t * 128:(t + 1) * 128],
                            KS[g][:, n * 512:(n + 1) * 512],
                            start=True, stop=True)
                nc.scalar.activation(
                    vh0[:], p01[:], mybir.ActivationFunctionType.Relu,
                    bias=bias8[:], scale=1.0,
                    accum_out=sv2[:, t, 0:1])
                nc.vector.tensor_tensor(
                    out=val[:, 0:1024], in0=vh0[:, 0:1024],
                    in1=vh0[:, 1024:2048], op=Alu.max)

                # half 1: ACT evacuates group 2 (own PSUM tile), DVE
                # evacuates group 1 fused with the merge (PSUM source, 1x)
                # + exact accum; separate tiles keep the two loops
                # independent.
                p0b = psum.tile([128, 1024], dt.float32, tag="p0b")
                p1b = psum.tile([128, 1024], dt.float32, tag="p1b")
                for g, pg in ((1, p1b), (0, p0b)):
                    for n in range(2):
                        nc.tensor.matmul(
                            pg[:, n * 512:(n + 1) * 512],
                            QS[g][:, t * 128:(t + 1) * 128],
                            KS[g][:, 1024 + n * 512:1024 + (n + 1) * 512],
                            start=True, stop=True)
                nc.scalar.activation(
                    v2h1[:], p1b[:], mybir.ActivationFunctionType.Relu,
                    bias=bias8[:], scale=1.0)
                val1 = vals.tile([128, 1024], dt.float16, tag="val1")
                nc.vector.scalar_tensor_tensor(
                    out=val1[:], in0=p0b[:], scalar=-8.0,
                    in1=v2h1[:], op0=Alu.add, op1=Alu.max,
                    accum_out=sv2[:, t, 1:2])

                # fold 2048 -> 256 (all-fp16 2x tt) and extract the top-8
                m1 = vals.tile([128, 1024], dt.float16, tag="m1")
                nc.vector.tensor_tensor(
                    out=m1[:], in0=val[:, 0:1024], in1=val1[:], op=Alu.max)
                m2 = vals.tile([128, 512], dt.float16, tag="m2")
                nc.vector.tensor_tensor(
                    out=m2[:], in0=m1[:, 0:512], in1=m1[:, 512:1024],
                    op=Alu.max)
                m3 = vals.tile([128, 256], dt.float16, tag="m3")
                nc.vector.tensor_tensor(
                    out=m3[:], in0=m2[:, 0:256], in1=m2[:, 256:512],
                    op=Alu.max)
                nc.vector.max(t8all[:, 8 * t:8 * t + 8], m3[:])

                if t == 3:
                    half_tail(0)
            half_tail(1)

    return nc


def _get_program():
    if "prog" not in _CACHE:
        nc = _build_program()
        if not nc.is_finalized():
            nc.finalize()  # Bacc: runs wait-splitting + reg-alloc passes
        _CACHE["prog"] = nc
    return _CACHE["prog"]


def _ramp_rows():
    """[2, L] bf16 rows summing (via the all-ones weight rows) to
    ramp(j) = (2048-j)*2^-13: hi = (128-(j>>4))*2^-9, lo = -(j&15)*2^-13.
    Every term is exactly representable in bf16, and relu(P-16) lands in
    (0, 0.25] where fp16 spacing is <= 2^-13, so values stay exact."""
    import ml_dtypes
    j = np.arange(L)
    hi = (128 - (j >> 4)).astype(np.float32) * 2.0 ** -9
    lo = -(j & 15).astype(np.float32) * 2.0 ** -13
    return np.stack([hi, lo]).astype(ml_dtypes.bfloat16)


def _make_in_maps(q, k):
    import ml_dtypes
    ramp = _ramp_rows()
    in_maps = []
    for c in range(N_CORES):
        b, h = divmod(c, 2)
        # bf16 rounding preserves (x > 0) for all reachable randn fp32
        qT = np.ascontiguousarray(
            q[b, h * QSH:(h + 1) * QSH, :].T.astype(ml_dtypes.bfloat16))
        kT = np.ascontiguousarray(k[b].T.astype(ml_dtypes.bfloat16))
        in_maps.append({"qT": qT, "kT": kT, "ramp": ramp})
    return in_maps


def run_device(q, k, trace=False):
    """Run the bass kernel on the 8 cores; returns (full_out, results_obj)."""
    from concourse.bass_utils import run_bass_kernel_spmd

    res = run_bass_kernel_spmd(
        _get_program(), _make_in_maps(q, k), list(range(N_CORES)), trace=trace)
    full = np.empty((B, L, K_MAX), np.int32)
    for c in range(N_CORES):
        b, h = divmod(c, 2)
        # out[p, 64t+c] = result for query row t*128+p
        blk = res.results[c]["out"].reshape(128, N_QT, K_MAX)
        full[b, h * QSH:(h + 1) * QSH, :] = (
            blk.transpose(1, 0, 2).reshape(QSH, K_MAX))
    return full, res


def _reference_numpy(q, k):
    """Exact numpy fallback (used only if some row has >= 8 matches)."""
    out = np.full((B, L, K_MAX), -1, np.int32)
    for b in range(B):
        qb = (q[b] > 0)
        kb = (k[b] > 0)
        match = np.zeros((L, L), bool)
        for lo in (0, 32):
            qg = qb[:, lo:lo + 32]
            kg = kb[:, lo:lo + 32]
            # pack 32 bits into one uint32 per row for exact equality
            qc = np.packbits(qg, axis=1).view(">u4").ravel()
            kc = np.packbits(kg, axis=1).view(">u4").ravel()
            match |= qc[:, None] == kc[None, :]
        for i in range(L):
            idx = np.nonzero(match[i])[0][:K_MAX]
            out[b, i, :len(idx)] = idx
    return out


def kernel(query_up, key_up, head_idx=None, **_unused):
    q = np.asarray(query_up, dtype=np.float32)
    k = np.asarray(key_up, dtype=np.float32)
    assert q.shape == (B, L, D) and k.shape == (B, L, D)
    full, _ = run_device(q, k)
    # Exact overflow detection: a non(-1) 8th candidate means the row had
    # >= 8 matches, so candidates 9.. might have been dropped.
    if (full[..., 7] != -1).any():
        full = _reference_numpy(q, k)
    return full


# revision 47
# speedup vs baseline: 1.3698x; 1.0488x over previous
"""Trainium2 Bass kernel for the CandidateFinder sparse-attention problem.

Computes, for each (batch, query) row, the first K_MAX=64 key indices whose
32-bit sign pattern exactly matches the query's in either of two dim groups
(dims 0:32, 32:64), padded with -1.

Approach (per core; 8 cores = 4 batches x 2 query halves):
  - inputs arrive as bf16 (sign-preserving host cast, halves DMA bytes);
    signs scaled to +-0.5 ((x>0) - 0.5 via one DVE tensor_scalar pass per
    group-tensor, exact, 4x mode on bf16); per group
    S_g[q,j]/4 = sum_d q_d k_d is a quarter-integer in [-8,8];
    match <=> S_g/4 == 8. (TensorE, K=34.)
  - two extra contraction rows add ramp(j) = (2048-j)*2^-13 (sum of two
    bf16-exact products), so P_g = S_g/4 + ramp is exact in fp32 PSUM and
    strictly decreasing in j for fixed S.
  - per query tile, the key axis is processed in two PSUM phases:
      h0 (keys 0:1024): both groups' matmuls land in one 4-bank PSUM tile;
      ONE 2048-wide ACT relu(P - 8) evacuates it to fp16 (matched
      positions give exactly (2048-j)*2^-13, rest 0), and its accumulator
      gives an exact-unless-2^-64 upper bound on the merged sum; a DVE
      all-fp16 tensor_tensor max (2x mode) merges the two groups.
      h1 (keys 1024:2048): the groups use separate PSUM tiles so their
      evacuations free them independently (PSUM dependencies are tracked
      per tile): ACT relus group 2 while DVE evacuates group 1 fused with
      the merge (scalar_tensor_tensor, accum_out = exact merged sum).
    ACT and DVE run near-balanced (~3.1us each per tile); PE warmup
    matmuls at t~0 lift the HAM clock gate, and a dependency-free ACT
    warmup pulls the ~1.3us activation-table load off the critical path.
  - three pairwise fp16 max folds shrink the row 2048 -> 256 before the
    DVE `max` (hardware top-8, descending) extracts the first <=8
    matching j; max never alters values, so survivors encode j exactly.
  - loss detection by sum conservation, batched per 4-tile half:
    sum(top8) == acc(h0) + acc(h1) iff no fold collision dropped a match
    and the row had <= 8 matches; flag = (sum > top8sum) forces a
    positive 8th slot, which triggers the exact host fallback.
  - three 2-source ops per half decode the top-8 values to j / -1 into a
    packed [128, 4*64] half of the output block; each half is written back
    with its own DMA so the first overlaps the second half's compute.
  - rows whose 8th candidate decodes as a real match (>=8 real matches, or
    the collision flag) are recomputed exactly on the host with numpy. With
    random normal inputs this never triggers: a match needs a 2^-32
    sign-pattern collision.

Self-contained: hardcodes shapes from the problem spec.
"""

import numpy as np

B = 4
L = 2048
D = 64
K_MAX = 64
N_CORES = 8
QSH = B * L // N_CORES  # 1024 queries per core
N_QT = QSH // 128       # 8 query tiles per core

_CACHE = {}


def _build_program(reps=1):
    from contextlib import ExitStack

    import concourse.bacc as bacc
    import concourse.mybir as mybir
    import concourse.tile as tile

    dt = mybir.dt
    Alu = mybir.AluOpType

    # Bacc (not raw Bass): its legalization passes split multi-sem waits,
    # which PE instructions can't carry (1 wait max per instruction).
    nc = bacc.Bacc("TRN2", target_bir_lowering=False, debug=False)
    # bf16 inputs: rounding fp32 -> bf16 preserves the sign bit, and bf16
    # flushes to 0.0 only below 1e-40, unreachable for randn fp32 data, so
    # (x > 0) is unchanged. Halves DMA bytes and lets the sign ops run in
    # the DVE's 4x perf mode.
    qT_d = nc.declare_dram_parameter("qT", [D, QSH], dt.bfloat16, isOutput=False)
    kT_d = nc.declare_dram_parameter("kT", [D, L], dt.bfloat16, isOutput=False)
    ramp_d = nc.declare_dram_parameter("ramp", [2, L], dt.bfloat16, isOutput=False)
    # packed output: out[p, 64*t + c] = candidate c of query row t*128 + p
    out_d = nc.declare_dram_parameter("out", [128, N_QT * K_MAX], dt.int32,
                                      isOutput=True)

    with tile.TileContext(nc) as tc, ExitStack() as ctx:
        consts = ctx.enter_context(tc.tile_pool(name="consts", bufs=1))
        vals = ctx.enter_context(tc.tile_pool(name="vals", bufs=3))
        outs = ctx.enter_context(tc.tile_pool(name="outs", bufs=2))
        psum = ctx.enter_context(tc.tile_pool(name="psum", bufs=1, space="PSUM"))

        # ---- load raw (transposed) inputs, split by dim group ----
        # all on SP (the ACT queue would stall a DMA setup behind the
        # activation table load); group 1 first — it gates the first signs
        qraw = consts.tile([D, QSH], dt.bfloat16)
        kraw = consts.tile([D, L], dt.bfloat16)
        nc.sync.dma_start(kraw[0:32, :], kT_d[0:32, :])
        nc.sync.dma_start(qraw[0:32, :], qT_d[0:32, :])
        nc.sync.dma_start(kraw[32:64, :], kT_d[32:64, :])
        nc.sync.dma_start(qraw[32:64, :], qT_d[32:64, :])

        # constants: all memsets/small DMAs on the otherwise-idle Pool
        # engine, ordered by when they are first needed (z64 first: it
        # feeds the PE warmup; the ramp DMAs gate the first matmuls so
        # they precede the slower ones-row memsets)
        z64 = consts.tile([128, 64], dt.float32, tag="z64")
        nc.gpsimd.memset(z64[:], 0.0)
        bias8 = consts.tile([128, 1], dt.float32, tag="bias8")
        nc.gpsimd.memset(bias8[:], -8.0)

        # ACT warmup: a dependency-free activation so the ~1.3us Relu
        # table load happens at t~0 instead of attaching itself (and its
        # successor's waits) to the first real evacuation.
        actw = consts.tile([128, 1], dt.float32, tag="actw")
        nc.scalar.activation(
            actw[:], z64[:, 0:1], mybir.ActivationFunctionType.Relu,
            bias=0.0, scale=1.0)

        # ---- sign tiles (+ ramp rows) ----
        # QS[g]: [34, QSH]  rows 0:32 = query signs (+-0.5), rows 32/33 = 1.0
        # KS[g]: [34, L]    rows 0:32 = key signs (+-0.5), rows 32/33 = ramp
        # All four sign passes on DVE: bf16 in/out, SBUF, step 1 -> 4x mode.
        # (x>0) - 0.5 -> +-0.5 exactly; x == 0 -> -0.5 like the reference.
        QS = []
        KS = []
        for g in range(2):
            qs = consts.tile([34, QSH], dt.bfloat16, tag=f"qs{g}")
            ks = consts.tile([34, L], dt.bfloat16, tag=f"ks{g}")
            QS.append(qs)
            KS.append(ks)
            # ramp terms (host-precomputed bf16 constants) into rows 32/33
            nc.gpsimd.dma_start(ks[32:34, :], ramp_d[:])
        for g in range(2):
            nc.gpsimd.memset(QS[g][32:34, :], 1.0)
        # key signs first: they are the longer pass and gate the first
        # matmul together with the query signs
        for g in range(2):
            lo, hi = g * 32, (g + 1) * 32
            nc.vector.tensor_scalar(
                out=KS[g][0:32, :], in0=kraw[lo:hi, :],
                scalar1=0.0, scalar2=0.5, op0=Alu.is_gt, op1=Alu.subtract)
            nc.vector.tensor_scalar(
                out=QS[g][0:32, :], in0=qraw[lo:hi, :],
                scalar1=0.0, scalar2=0.5, op0=Alu.is_gt, op1=Alu.subtract)

        c2048 = consts.tile([128, 64], dt.float32, tag="c2048")
        nc.gpsimd.memset(c2048[:], 2048.0)
        # all 8 query tiles' top-8 values, decoded per 4-tile half
        t8all = consts.tile([128, 64], dt.float16, tag="t8all")
        # per-tile exact sums: col 2t = Pool accum of the merged h0 block,
        # col 2t+1 = DVE accum of the merged h1 block (both exact: a
        # both-group match contributes once, like the reference's union)
        svall = consts.tile([128, 2 * N_QT], dt.float32, tag="svall")
        # packed output block; -1 everywhere the decode doesn't overwrite
        o2 = consts.tile([128, N_QT * K_MAX], dt.int32, tag="o2")
        nc.gpsimd.memset(o2[:], -1)

        # ---- PE warmup: ~10 zero matmuls so the HAM clock gate reaches
        # full speed before the first real matmuls (the activity window is
        # ~3.4us; these run while the DMAs and signs are still in flight).
        # z64 is all zeros so the scratch PSUM results are finite; they
        # reuse the p01 storage, which tile 0 overwrites with start=True.
        pwarm = psum.tile([128, 1024], dt.float32, tag="p0b")
        for w in range(10):
            nc.tensor.matmul(
                pwarm[0:64, w * 64:w * 64 + 64],
                z64[:], z64[:, 0:64], start=True, stop=True)

        t8v = t8all.rearrange("p (t c) -> p t c", c=8)
        sv2 = svall.rearrange("p (t c) -> p t c", c=2)
        o2v = o2.rearrange("p (t c) -> p t c", c=K_MAX)

        def half_tail(h):
            """Flags + decode + writeback for tiles 4h..4h+3."""
            ts = slice(4 * h, 4 * h + 4)
            # sum(val) from the two exact merged-block accums
            up4 = outs.tile([128, 4], dt.float32, tag="up4")
            nc.vector.tensor_tensor(
                out=up4[:], in0=sv2[:, ts, 0], in1=sv2[:, ts, 1], op=Alu.add)
            # sum of the extracted top-8 values (exact in fp32)
            ts4 = outs.tile([128, 4], dt.float32, tag="ts4")
            nc.vector.tensor_reduce(
                out=ts4[:], in_=t8v[:, ts, :], axis=mybir.AxisListType.X,
                op=Alu.add)
            # flag = some match was dropped (fold collision, > 8 matches, or
            # a 2^-64 both-group match) -> force slot 7 positive
            fl4 = outs.tile([128, 4], dt.float32, tag="fl4")
            nc.vector.tensor_tensor(
                out=fl4[:], in0=up4[:], in1=ts4[:], op=Alu.is_gt)
            nc.vector.scalar_tensor_tensor(
                out=t8v[:, ts, 7], in0=fl4[:], scalar=2.0 ** -13,
                in1=t8v[:, ts, 7], op0=Alu.mult, op1=Alu.max)
            # decode: matched v = (2048-j)*2^-13 => u = 2048 - 8192*v = j;
            # unmatched v = 0 => u = 2048 -> -1.
            cols = slice(32 * h, 32 * h + 32)
            u = outs.tile([128, 32], dt.float32, tag="u")
            nc.vector.scalar_tensor_tensor(
                out=u[:], in0=t8all[:, cols], scalar=-8192.0,
                in1=c2048[:, 0:32], op0=Alu.mult, op1=Alu.add)
            pad = outs.tile([128, 32], dt.float32, tag="pad")
            nc.vector.scalar_tensor_tensor(
                out=pad[:], in0=u[:], scalar=-2047.0, in1=z64[:, 0:32],
                op0=Alu.add, op1=Alu.max)
            # o = u - 2049*pad -> j or -1 (int32 cast on write), scattered
            # into the first 8 columns of each tile's 64-column block
            nc.vector.scalar_tensor_tensor(
                out=o2v[:, ts, 0:8],
                in0=pad.rearrange("p (t c) -> p t c", c=8),
                scalar=-2049.0,
                in1=u.rearrange("p (t c) -> p t c", c=8),
                op0=Alu.mult, op1=Alu.add)
            # writeback from SP (idle by now; cheaper DGE setup than SWDGE)
            nc.sync.dma_start(out_d[:, 256 * h:256 * (h + 1)],
                              o2[:, 256 * h:256 * (h + 1)])

        # ---- main loop over query tiles ----
        # reps>1 repeats the whole body inside one NEFF (timing only).
        for _ in range(reps):
            for t in range(N_QT):
                val = vals.tile([128, 1024], dt.float16, tag="val")
                vh0 = vals.tile([128, 2048], dt.float16, tag="vh0")
                v2h1 = vals.tile([128, 1024], dt.float16, tag="v2h1")

                # half 0: both groups' matmuls land in one 4-bank PSUM tile
                # so ONE 2048-wide ACT relu evacuates both; its accumulator
                # gives sum(vh0) >= sum(val_h0), tight unless a 2^-64
                # both-group match (which then just false-positives the
                # exact host fallback). DVE merges with one all-fp16 2x tt.
                p01 = psum.tile([128, 2048], dt.float32, tag="p01")
                for g in range(2):
                    for n in range(2):
                        nc.tensor.matmul(
                            p01[:, g * 1024 + n * 512:g * 1024 + (n + 1) * 512],
                            QS[g][:, t * 128:(t + 1) * 128],
                            KS[g][:, n * 512:(n + 1) * 512],
                            start=True, stop=True)
                nc.scalar.activation(
                    vh0[:], p01[:], mybir.ActivationFunctionType.Relu,
                    bias=bias8[:], scale=1.0,
                    accum_out=sv2[:, t, 0:1])
                nc.vector.tensor_tensor(
                    out=val[:, 0:1024], in0=vh0[:, 0:1024],
                    in1=vh0[:, 1024:2048], op=Alu.max)

                # half 1: ACT evacuates group 2 (own PSUM tile), DVE
                # evacuates group 1 fused with the merge (PSUM source, 1x)
                # + exact accum; separate tiles keep the two loops
                # independent.
                p0b = psum.tile([128, 1024], dt.float32, tag="p0b")
                p1b = psum.tile([128, 1024], dt.float32, tag="p1b")
                for g, pg in ((1, p1b), (0, p0b)):
                    for n in range(2):
                        nc.tensor.matmul(
                            pg[:, n * 512:(n + 1) * 512],
                            QS[g][:, t * 128:(t + 1) * 128],
                            KS[g][:, 1024 + n * 512:1024 + (n + 1) * 512],
                            start=True, stop=True)
                nc.scalar.activation(
                    v2h1[:], p1b[:], mybir.ActivationFunctionType.Relu,
                    bias=bias8[:], scale=1.0)
                val1 = vals.tile([128, 1024], dt.float16, tag="val1")
                nc.vector.scalar_tensor_tensor(
                    out=val1[:], in0=p0b[:], scalar=-8.0,
                    in1=v2h1[:], op0=Alu.add, op1=Alu.max,
                    accum_out=sv2[:, t, 1:2])

                # fold 2048 -> 256 (all-fp16 2x tt) and extract the top-8
                m1 = vals.tile([128, 1024], dt.float16, tag="m1")
                nc.vector.tensor_tensor(
                    out=m1[:], in0=val[:, 0:1024], in1=val1[:], op=Alu.max)
                m2 = vals.tile([128, 512], dt.float16, tag="m2")
                nc.vector.tensor_tensor(
                    out=m2[:], in0=m1[:, 0:512], in1=m1[:, 512:1024],
                    op=Alu.max)
                m3 = vals.tile([128, 256], dt.float16, tag="m3")
                nc.vector.tensor_tensor(
                    out=m3[:], in0=m2[:, 0:256], in1=m2[:, 256:512],
                    op=Alu.max)
                nc.vector.max(t8all[:, 8 * t:8 * t + 8], m3[:])

                if t == 3:
                    half_tail(0)
            half_tail(1)

    return nc


def _get_program():
    if "prog" not in _CACHE:
        nc = _build_program()
        if not nc.is_finalized():
            nc.finalize()  # Bacc: runs wait-splitting + reg-alloc passes
        _CACHE["prog"] = nc
    return _CACHE["prog"]


def _ramp_rows():
    """[2, L] bf16 rows summing (via the all-ones weight rows) to
    ramp(j) = (2048-j)*2^-13: hi = (128-(j>>4))*2^-9, lo = -(j&15)*2^-13.
    Every term is exactly representable in bf16, and relu(P-16) lands in
    (0, 0.25] where fp16 spacing is <= 2^-13, so values stay exact."""
    import ml_dtypes
    j = np.arange(L)
    hi = (128 - (j >> 4)).astype(np.float32) * 2.0 ** -9
    lo = -(j & 15).astype(np.float32) * 2.0 ** -13
    return np.stack([hi, lo]).astype(ml_dtypes.bfloat16)


def _make_in_maps(q, k):
    import ml_dtypes
    ramp = _ramp_rows()
    in_maps = []
    for c in range(N_CORES):
        b, h = divmod(c, 2)
        # bf16 rounding preserves (x > 0) for all reachable randn fp32
        qT = np.ascontiguousarray(
            q[b, h * QSH:(h + 1) * QSH, :].T.astype(ml_dtypes.bfloat16))
        kT = np.ascontiguousarray(k[b].T.astype(ml_dtypes.bfloat16))
        in_maps.append({"qT": qT, "kT": kT, "ramp": ramp})
    return in_maps


def run_device(q, k, trace=False):
    """Run the bass kernel on the 8 cores; returns (full_out, results_obj)."""
    from concourse.bass_utils import run_bass_kernel_spmd

    res = run_bass_kernel_spmd(
        _get_program(), _make_in_maps(q, k), list(range(N_CORES)), trace=trace)
    full = np.empty((B, L, K_MAX), np.int32)
    for c in range(N_CORES):
        b, h = divmod(c, 2)
        # out[p, 64t+c] = result for query row t*128+p
        blk = res.results[c]["out"].reshape(128, N_QT, K_MAX)
        full[b, h * QSH:(h + 1) * QSH, :] = (
            blk.transpose(1, 0, 2).reshape(QSH, K_MAX))
    return full, res


def _reference_numpy(q, k):
    """Exact numpy fallback (used only if some row has >= 8 matches)."""
    out = np.full((B, L, K_MAX), -1, np.int32)
    for b in range(B):
        qb = (q[b] > 0)
        kb = (k[b] > 0)
        match = np.zeros((L, L), bool)
        for lo in (0, 32):
            qg = qb[:, lo:lo + 32]
            kg = kb[:, lo:lo + 32]
            # pack 32 bits into one uint32 per row for exact equality
            qc = np.packbits(qg, axis=1).view(">u4").ravel()
            kc = np.packbits(kg, axis=1).view(">u4").ravel()
            match |= qc[:, None] == kc[None, :]
        for i in range(L):
            idx = np.nonzero(match[i])[0][:K_MAX]
            out[b, i, :len(idx)] = idx
    return out


def kernel(query_up, key_up, head_idx=None, **_unused):
    q = np.asarray(query_up, dtype=np.float32)
    k = np.asarray(key_up, dtype=np.float32)
    assert q.shape == (B, L, D) and k.shape == (B, L, D)
    full, _ = run_device(q, k)
    # Exact overflow detection: a non(-1) 8th candidate means the row had
    # >= 8 matches, so candidates 9.. might have been dropped.
    if (full[..., 7] != -1).any():
        full = _reference_numpy(q, k)
    return full


# revision 49
# speedup vs baseline: 1.5765x; 1.1509x over previous
"""Trainium2 Bass kernel for the CandidateFinder sparse-attention problem.

Computes, for each (batch, query) row, the first K_MAX=64 key indices whose
32-bit sign pattern exactly matches the query's in either of two dim groups
(dims 0:32, 32:64), padded with -1.

Approach (per core; 8 cores = 4 batches x 2 query halves):
  - inputs arrive as bf16 (sign-preserving host cast, halves DMA bytes);
    signs scaled to +-0.5 ((x>0) - 0.5 via one DVE tensor_scalar pass per
    group-tensor, exact, 4x mode on bf16); per group
    S_g[q,j]/4 = sum_d q_d k_d is a quarter-integer in [-8,8];
    match <=> S_g/4 == 8. (TensorE, K=34.)
  - two extra contraction rows add ramp(j) = (2048-j)*2^-13 (sum of two
    bf16-exact products), so P_g = S_g/4 + ramp is exact in fp32 PSUM and
    strictly decreasing in j for fixed S.
  - per query tile, the key axis is processed in two PSUM phases:
      h0 (keys 0:1024): both groups' matmuls land in one 4-bank PSUM tile;
      ONE 2048-wide ACT relu(P - 8) evacuates it to fp16 (matched
      positions give exactly (2048-j)*2^-13, rest 0), and its accumulator
      gives an exact-unless-2^-64 upper bound on the merged sum; a DVE
      all-fp16 tensor_tensor max (2x mode) merges the two groups.
      h1 (keys 1024:2048): the groups use separate PSUM tiles so their
      evacuations free them independently (PSUM dependencies are tracked
      per tile): ACT relus group 2 while DVE evacuates group 1 fused with
      the merge (scalar_tensor_tensor, accum_out = exact merged sum).
    ACT and DVE run near-balanced (~3.1us each per tile); PE warmup
    matmuls at t~0 lift the HAM clock gate, and a dependency-free ACT
    warmup pulls the ~1.3us activation-table load off the critical path.
  - three pairwise fp16 max folds shrink the row 2048 -> 256 before the
    DVE `max` (hardware top-8, descending) extracts the first <=8
    matching j; max never alters values, so survivors encode j exactly.
  - loss detection by sum conservation, batched per 4-tile half:
    sum(top8) == acc(h0) + acc(h1) iff no fold collision dropped a match
    and the row had <= 8 matches; flag = (sum > top8sum) forces a
    positive 8th slot, which triggers the exact host fallback.
  - three 2-source ops per half decode the top-8 values to j / -1 into a
    packed [128, 4*64] half of the output block; each half is written back
    with its own DMA so the first overlaps the second half's compute.
  - rows whose 8th candidate decodes as a real match (>=8 real matches, or
    the collision flag) are recomputed exactly on the host with numpy. With
    random normal inputs this never triggers: a match needs a 2^-32
    sign-pattern collision.

Self-contained: hardcodes shapes from the problem spec.
"""

import numpy as np

B = 4
L = 2048
D = 64
K_MAX = 64
N_CORES = 8
QSH = B * L // N_CORES  # 1024 queries per core
N_QT = QSH // 128       # 8 query tiles per core

_CACHE = {}


def _build_program(reps=1):
    from contextlib import ExitStack

    import concourse.bacc as bacc
    import concourse.mybir as mybir
    import concourse.tile as tile

    dt = mybir.dt
    Alu = mybir.AluOpType

    # Bacc (not raw Bass): its legalization passes split multi-sem waits,
    # which PE instructions can't carry (1 wait max per instruction).
    nc = bacc.Bacc("TRN2", target_bir_lowering=False, debug=False)
    # bf16 inputs: rounding fp32 -> bf16 preserves the sign bit, and bf16
    # flushes to 0.0 only below 1e-40, unreachable for randn fp32 data, so
    # (x > 0) is unchanged. Halves DMA bytes and lets the sign ops run in
    # the DVE's 4x perf mode.
    qT_d = nc.declare_dram_parameter("qT", [D, QSH], dt.bfloat16, isOutput=False)
    kT_d = nc.declare_dram_parameter("kT", [D, L], dt.bfloat16, isOutput=False)
    ramp_d = nc.declare_dram_parameter("ramp", [2, L], dt.bfloat16, isOutput=False)
    # packed output: out[p, 64*t + c] = candidate c of query row t*128 + p
    out_d = nc.declare_dram_parameter("out", [128, N_QT * K_MAX], dt.int32,
                                      isOutput=True)

    with tile.TileContext(nc) as tc, ExitStack() as ctx:
        consts = ctx.enter_context(tc.tile_pool(name="consts", bufs=1))
        vals = ctx.enter_context(tc.tile_pool(name="vals", bufs=3))
        outs = ctx.enter_context(tc.tile_pool(name="outs", bufs=2))
        psum = ctx.enter_context(tc.tile_pool(name="psum", bufs=1, space="PSUM"))

        # ---- load raw (transposed) inputs, split by dim group ----
        # all on SP (the ACT queue would stall a DMA setup behind the
        # activation table load); group 1 first — it gates the first signs
        qraw = consts.tile([D, QSH], dt.bfloat16)
        kraw = consts.tile([D, L], dt.bfloat16)
        nc.sync.dma_start(kraw[0:32, :], kT_d[0:32, :])
        nc.sync.dma_start(qraw[0:32, :], qT_d[0:32, :])
        nc.sync.dma_start(kraw[32:64, :], kT_d[32:64, :])
        nc.sync.dma_start(qraw[32:64, :], qT_d[32:64, :])

        # constants: all memsets/small DMAs on the otherwise-idle Pool
        # engine, ordered by when they are first needed (z64 first: it
        # feeds the PE warmup; the ramp DMAs gate the first matmuls so
        # they precede the slower ones-row memsets)
        z64 = consts.tile([128, 64], dt.float32, tag="z64")
        nc.gpsimd.memset(z64[:], 0.0)
        bias8 = consts.tile([128, 1], dt.float32, tag="bias8")
        nc.gpsimd.memset(bias8[:], -8.0)

        # ACT warmup: a dependency-free activation so the ~1.3us Relu
        # table load happens at t~0 instead of attaching itself (and its
        # successor's waits) to the first real evacuation.
        actw = consts.tile([128, 1], dt.float32, tag="actw")
        nc.scalar.activation(
            actw[:], z64[:, 0:1], mybir.ActivationFunctionType.Relu,
            bias=0.0, scale=1.0)

        # ---- sign tiles (+ ramp rows) ----
        # QS[g]: [34, QSH]  rows 0:32 = query signs (+-0.5), rows 32/33 = 1.0
        # KS[g]: [34, L]    rows 0:32 = key signs (+-0.5), rows 32/33 = ramp
        # All four sign passes on DVE: bf16 in/out, SBUF, step 1 -> 4x mode.
        # (x>0) - 0.5 -> +-0.5 exactly; x == 0 -> -0.5 like the reference.
        QS = []
        KS = []
        for g in range(2):
            qs = consts.tile([34, QSH], dt.bfloat16, tag=f"qs{g}")
            ks = consts.tile([34, L], dt.bfloat16, tag=f"ks{g}")
            QS.append(qs)
            KS.append(ks)
            # ramp terms (host-precomputed bf16 constants) into rows 32/33
            nc.gpsimd.dma_start(ks[32:34, :], ramp_d[:])
        for g in range(2):
            nc.gpsimd.memset(QS[g][32:34, :], 1.0)
        # key signs first: they are the longer pass and gate the first
        # matmul together with the query signs
        for g in range(2):
            lo, hi = g * 32, (g + 1) * 32
            nc.vector.tensor_scalar(
                out=KS[g][0:32, :], in0=kraw[lo:hi, :],
                scalar1=0.0, scalar2=0.5, op0=Alu.is_gt, op1=Alu.subtract)
            nc.vector.tensor_scalar(
                out=QS[g][0:32, :], in0=qraw[lo:hi, :],
                scalar1=0.0, scalar2=0.5, op0=Alu.is_gt, op1=Alu.subtract)

        c2048 = consts.tile([128, 64], dt.float32, tag="c2048")
        nc.gpsimd.memset(c2048[:], 2048.0)
        # all 8 query tiles' top-8 values, decoded per 4-tile half
        t8all = consts.tile([128, 64], dt.float16, tag="t8all")
        # per-tile exact sums: col 2t = Pool accum of the merged h0 block,
        # col 2t+1 = DVE accum of the merged h1 block (both exact: a
        # both-group match contributes once, like the reference's union)
        # (+2 spare cols: tile 0 splits its h0 evacuation for an earlier
        # pipeline start, so its h0 sum arrives as two accums)
        svall = consts.tile([128, 2 * N_QT + 2], dt.float32, tag="svall")
        # packed output block; -1 everywhere the decode doesn't overwrite
        o2 = consts.tile([128, N_QT * K_MAX], dt.int32, tag="o2")
        nc.gpsimd.memset(o2[:], -1)

        # ---- PE warmup: ~10 zero matmuls so the HAM clock gate reaches
        # full speed before the first real matmuls (the activity window is
        # ~3.4us; these run while the DMAs and signs are still in flight).
        # z64 is all zeros so the scratch PSUM results are finite; they
        # reuse the p01 storage, which tile 0 overwrites with start=True.
        pwarm = psum.tile([128, 1024], dt.float32, tag="p0b")
        for w in range(10):
            nc.tensor.matmul(
                pwarm[0:64, w * 64:w * 64 + 64],
                z64[:], z64[:, 0:64], start=True, stop=True)

        t8v = t8all.rearrange("p (t c) -> p t c", c=8)
        sv2 = svall[:, 0:2 * N_QT].rearrange("p (t c) -> p t c", c=2)
        o2v = o2.rearrange("p (t c) -> p t c", c=K_MAX)

        def tail(t0, nt, extra_accum=False):
            """Flags + decode + writeback for tiles t0..t0+nt-1."""
            ts = slice(t0, t0 + nt)
            # sum(val) from the two exact merged-block accums
            up4 = outs.tile([128, 4], dt.float32, tag="up4")
            nc.vector.tensor_tensor(
                out=up4[:, 0:nt], in0=sv2[:, ts, 0], in1=sv2[:, ts, 1],
                op=Alu.add)
            if extra_accum:
                # tile 0's second h0 accum (split evacuation)
                nc.vector.tensor_tensor(
                    out=up4[:, 0:1], in0=up4[:, 0:1],
                    in1=svall[:, 2 * N_QT:2 * N_QT + 1], op=Alu.add)
            # sum of the extracted top-8 values (exact in fp32)
            ts4 = outs.tile([128, 4], dt.float32, tag="ts4")
            nc.vector.tensor_reduce(
                out=ts4[:, 0:nt], in_=t8v[:, ts, :], axis=mybir.AxisListType.X,
                op=Alu.add)
            # flag = some match was dropped (fold collision, > 8 matches, or
            # a 2^-64 both-group match) -> force slot 7 positive
            fl4 = outs.tile([128, 4], dt.float32, tag="fl4")
            nc.vector.tensor_tensor(
                out=fl4[:, 0:nt], in0=up4[:, 0:nt], in1=ts4[:, 0:nt],
                op=Alu.is_gt)
            nc.vector.scalar_tensor_tensor(
                out=t8v[:, ts, 7], in0=fl4[:, 0:nt], scalar=2.0 ** -13,
                in1=t8v[:, ts, 7], op0=Alu.mult, op1=Alu.max)
            # decode: matched v = (2048-j)*2^-13 => u = 2048 - 8192*v = j;
            # unmatched v = 0 => u = 2048 -> -1.
            w = 8 * nt
            cols = slice(8 * t0, 8 * t0 + w)
            u = outs.tile([128, 32], dt.float32, tag="u")
            nc.vector.scalar_tensor_tensor(
                out=u[:, 0:w], in0=t8all[:, cols], scalar=-8192.0,
                in1=c2048[:, 0:w], op0=Alu.mult, op1=Alu.add)
            pad = outs.tile([128, 32], dt.float32, tag="pad")
            nc.vector.scalar_tensor_tensor(
                out=pad[:, 0:w], in0=u[:, 0:w], scalar=-2047.0,
                in1=z64[:, 0:w], op0=Alu.add, op1=Alu.max)
            # o = u - 2049*pad -> j or -1 (int32 cast on write), scattered
            # into the first 8 columns of each tile's 64-column block
            nc.vector.scalar_tensor_tensor(
                out=o2v[:, ts, 0:8],
                in0=pad[:, 0:w].rearrange("p (t c) -> p t c", c=8),
                scalar=-2049.0,
                in1=u[:, 0:w].rearrange("p (t c) -> p t c", c=8),
                op0=Alu.mult, op1=Alu.add)
            # writeback from SP (idle by now; cheaper DGE setup than SWDGE)
            nc.sync.dma_start(out_d[:, 64 * t0:64 * (t0 + nt)],
                              o2[:, 64 * t0:64 * (t0 + nt)])

        # ---- main loop over query tiles ----
        # reps>1 repeats the whole body inside one NEFF (timing only).
        for _ in range(reps):
            for t in range(N_QT):
                val = vals.tile([128, 1024], dt.float16, tag="val")
                vh0 = vals.tile([128, 2048], dt.float16, tag="vh0")
                v2h1 = vals.tile([128, 1024], dt.float16, tag="v2h1")

                # half 0: both groups' matmuls land in one 4-bank PSUM tile
                # so ONE 2048-wide ACT relu evacuates both; its accumulator
                # gives sum(vh0) >= sum(val_h0), tight unless a 2^-64
                # both-group match (which then just false-positives the
                # exact host fallback). DVE merges with one all-fp16 2x tt.
                p01 = psum.tile([128, 2048], dt.float32, tag="p01")
                for g in range(2):
                    for n in range(2):
                        nc.tensor.matmul(
                            p01[:, g * 1024 + n * 512:g * 1024 + (n + 1) * 512],
                            QS[g][:, t * 128:(t + 1) * 128],
                            KS[g][:, n * 512:(n + 1) * 512],
                            start=True, stop=True)
                if t == 0:
                    # tile 0 only: split the wide evacuation so ACT starts
                    # right after the g1 matmuls instead of waiting for all
                    # four chunks (the two accums are summed in the tail)
                    nc.scalar.activation(
                        vh0[:, 0:1024], p01[:, 0:1024],
                        mybir.ActivationFunctionType.Relu,
                        bias=bias8[:], scale=1.0,
                        accum_out=sv2[:, t, 0:1])
                    nc.scalar.activation(
                        vh0[:, 1024:2048], p01[:, 1024:2048],
                        mybir.ActivationFunctionType.Relu,
                        bias=bias8[:], scale=1.0,
                        accum_out=svall[:, 2 * N_QT:2 * N_QT + 1])
                else:
                    nc.scalar.activation(
                        vh0[:], p01[:], mybir.ActivationFunctionType.Relu,
                        bias=bias8[:], scale=1.0,
                        accum_out=sv2[:, t, 0:1])
                nc.vector.tensor_tensor(
                    out=val[:, 0:1024], in0=vh0[:, 0:1024],
                    in1=vh0[:, 1024:2048], op=Alu.max)

                # half 1: ACT evacuates group 2 (own PSUM tile), DVE
                # evacuates group 1 fused with the merge (PSUM source, 1x)
                # + exact accum; separate tiles keep the two loops
                # independent.
                p0b = psum.tile([128, 1024], dt.float32, tag="p0b")
                p1b = psum.tile([128, 1024], dt.float32, tag="p1b")
                for g, pg in ((1, p1b), (0, p0b)):
                    for n in range(2):
                        nc.tensor.matmul(
                            pg[:, n * 512:(n + 1) * 512],
                            QS[g][:, t * 128:(t + 1) * 128],
                            KS[g][:, 1024 + n * 512:1024 + (n + 1) * 512],
                            start=True, stop=True)
                nc.scalar.activation(
                    v2h1[:], p1b[:], mybir.ActivationFunctionType.Relu,
                    bias=bias8[:], scale=1.0)
                val1 = vals.tile([128, 1024], dt.float16, tag="val1")
                nc.vector.scalar_tensor_tensor(
                    out=val1[:], in0=p0b[:], scalar=-8.0,
                    in1=v2h1[:], op0=Alu.add, op1=Alu.max,
                    accum_out=sv2[:, t, 1:2])

                # fold 2048 -> 256 (all-fp16 2x tt) and extract the top-8
                m1 = vals.tile([128, 1024], dt.float16, tag="m1")
                nc.vector.tensor_tensor(
                    out=m1[:], in0=val[:, 0:1024], in1=val1[:], op=Alu.max)
                m2 = vals.tile([128, 512], dt.float16, tag="m2")
                nc.vector.tensor_tensor(
                    out=m2[:], in0=m1[:, 0:512], in1=m1[:, 512:1024],
                    op=Alu.max)
                m3 = vals.tile([128, 256], dt.float16, tag="m3")
                nc.vector.tensor_tensor(
                    out=m3[:], in0=m2[:, 0:256], in1=m2[:, 256:512],
                    op=Alu.max)
                nc.vector.max(t8all[:, 8 * t:8 * t + 8], m3[:])

                if t == 3:
                    tail(0, 4, extra_accum=True)
                elif t == 6:
                    tail(4, 3)
            tail(7, 1)

    return nc


def _get_program():
    if "prog" not in _CACHE:
        nc = _build_program()
        if not nc.is_finalized():
            nc.finalize()  # Bacc: runs wait-splitting + reg-alloc passes
        _CACHE["prog"] = nc
    return _CACHE["prog"]


def _ramp_rows():
    """[2, L] bf16 rows summing (via the all-ones weight rows) to
    ramp(j) = (2048-j)*2^-13: hi = (128-(j>>4))*2^-9, lo = -(j&15)*2^-13.
    Every term is exactly representable in bf16, and relu(P-16) lands in
    (0, 0.25] where fp16 spacing is <= 2^-13, so values stay exact."""
    import ml_dtypes
    j = np.arange(L)
    hi = (128 - (j >> 4)).astype(np.float32) * 2.0 ** -9
    lo = -(j & 15).astype(np.float32) * 2.0 ** -13
    return np.stack([hi, lo]).astype(ml_dtypes.bfloat16)


def _make_in_maps(q, k):
    import ml_dtypes
    ramp = _ramp_rows()
    in_maps = []
    for c in range(N_CORES):
        b, h = divmod(c, 2)
        # bf16 rounding preserves (x > 0) for all reachable randn fp32
        qT = np.ascontiguousarray(
            q[b, h * QSH:(h + 1) * QSH, :].T.astype(ml_dtypes.bfloat16))
        kT = np.ascontiguousarray(k[b].T.astype(ml_dtypes.bfloat16))
        in_maps.append({"qT": qT, "kT": kT, "ramp": ramp})
    return in_maps


def run_device(q, k, trace=False):
    """Run the bass kernel on the 8 cores; returns (full_out, results_obj)."""
    from concourse.bass_utils import run_bass_kernel_spmd

    res = run_bass_kernel_spmd(
        _get_program(), _make_in_maps(q, k), list(range(N_CORES)), trace=trace)
    full = np.empty((B, L, K_MAX), np.int32)
    for c in range(N_CORES):
        b, h = divmod(c, 2)
        # out[p, 64t+c] = result for query row t*128+p
        blk = res.results[c]["out"].reshape(128, N_QT, K_MAX)
        full[b, h * QSH:(h + 1) * QSH, :] = (
            blk.transpose(1, 0, 2).reshape(QSH, K_MAX))
    return full, res


def _reference_numpy(q, k):
    """Exact numpy fallback (used only if some row has >= 8 matches)."""
    out = np.full((B, L, K_MAX), -1, np.int32)
    for b in range(B):
        qb = (q[b] > 0)
        kb = (k[b] > 0)
        match = np.zeros((L, L), bool)
        for lo in (0, 32):
            qg = qb[:, lo:lo + 32]
            kg = kb[:, lo:lo + 32]
            # pack 32 bits into one uint32 per row for exact equality
            qc = np.packbits(qg, axis=1).view(">u4").ravel()
            kc = np.packbits(kg, axis=1).view(">u4").ravel()
            match |= qc[:, None] == kc[None, :]
        for i in range(L):
            idx = np.nonzero(match[i])[0][:K_MAX]
            out[b, i, :len(idx)] = idx
    return out


def kernel(query_up, key_up, head_idx=None, **_unused):
    q = np.asarray(query_up, dtype=np.float32)
    k = np.asarray(key_up, dtype=np.float32)
    assert q.shape == (B, L, D) and k.shape == (B, L, D)
    full, _ = run_device(q, k)
    # Exact overflow detection: a non(-1) 8th candidate means the row had
    # >= 8 matches, so candidates 9.. might have been dropped.
    if (full[..., 7] != -1).any():
        full = _reference_numpy(q, k)
    return full


# revision 52
# speedup vs baseline: 1.9980x; 1.2674x over previous
"""Trainium2 Bass kernel for the CandidateFinder sparse-attention problem.

Computes, for each (batch, query) row, the first K_MAX=64 key indices whose
32-bit sign pattern exactly matches the query's in either of two dim groups
(dims 0:32, 32:64), padded with -1.

Approach (per core; 8 cores = 4 batches x 2 query halves):
  - inputs arrive as bf16 (sign-preserving host cast, halves DMA bytes);
    signs scaled to +-0.5 ((x>0) - 0.5 via one DVE tensor_scalar pass per
    group-tensor, exact, 4x mode on bf16); per group
    S_g[q,j]/4 = sum_d q_d k_d is a quarter-integer in [-8,8];
    match <=> S_g/4 == 8. (TensorE, K=34.)
  - two extra contraction rows add ramp(j) = (2048-j)*2^-13 (sum of two
    bf16-exact products), so P_g = S_g/4 + ramp is exact in fp32 PSUM and
    strictly decreasing in j for fixed S.
  - per query tile, the key axis is processed in two PSUM phases:
      h0 (keys 0:1024): both groups' matmuls land in one 4-bank PSUM tile;
      ONE 2048-wide ACT relu(P - 8) evacuates it to fp16 (matched
      positions give exactly (2048-j)*2^-13, rest 0), and its accumulator
      gives an exact-unless-2^-64 upper bound on the merged sum; a DVE
      all-fp16 tensor_tensor max (2x mode) merges the two groups.
      h1 (keys 1024:2048): the groups use separate PSUM tiles so their
      evacuations free them independently (PSUM dependencies are tracked
      per tile): ACT relus group 2 while DVE evacuates group 1 fused with
      the merge (scalar_tensor_tensor, accum_out = exact merged sum).
    ACT and DVE run near-balanced (~3.1us each per tile); PE warmup
    matmuls at t~0 lift the HAM clock gate, and a dependency-free ACT
    warmup pulls the ~1.3us activation-table load off the critical path.
  - three pairwise fp16 max folds shrink the row 2048 -> 256 before the
    DVE `max` (hardware top-8, descending) extracts the first <=8
    matching j; max never alters values, so survivors encode j exactly.
  - loss detection by sum conservation, batched per 4-tile half:
    sum(top8) == acc(h0) + acc(h1) iff no fold collision dropped a match
    and the row had <= 8 matches; flag = (sum > top8sum) forces a
    positive 8th slot, which triggers the exact host fallback.
  - three 2-source ops per half decode the top-8 values to j / -1 into a
    packed [128, 4*64] half of the output block; each half is written back
    with its own DMA so the first overlaps the second half's compute.
  - rows whose 8th candidate decodes as a real match (>=8 real matches, or
    the collision flag) are recomputed exactly on the host with numpy. With
    random normal inputs this never triggers: a match needs a 2^-32
    sign-pattern collision.

Self-contained: hardcodes shapes from the problem spec.
"""

import numpy as np

B = 4
L = 2048
D = 64
K_MAX = 64
N_CORES = 8
QSH = B * L // N_CORES  # 1024 queries per core
N_QT = QSH // 128       # 8 query tiles per core

_CACHE = {}


def _build_program(reps=1):
    from contextlib import ExitStack

    import concourse.bacc as bacc
    import concourse.mybir as mybir
    import concourse.tile as tile

    dt = mybir.dt
    Alu = mybir.AluOpType

    # Bacc (not raw Bass): its legalization passes split multi-sem waits,
    # which PE instructions can't carry (1 wait max per instruction).
    nc = bacc.Bacc("TRN2", target_bir_lowering=False, debug=False)
    # bf16 inputs: rounding fp32 -> bf16 preserves the sign bit, and bf16
    # flushes to 0.0 only below 1e-40, unreachable for randn fp32 data, so
    # (x > 0) is unchanged. Halves DMA bytes and lets the sign ops run in
    # the DVE's 4x perf mode.
    qT_d = nc.declare_dram_parameter("qT", [D, QSH], dt.bfloat16, isOutput=False)
    kT_d = nc.declare_dram_parameter("kT", [D, L], dt.bfloat16, isOutput=False)
    ramp_d = nc.declare_dram_parameter("ramp", [2, L], dt.bfloat16, isOutput=False)
    # packed output: out[p, 64*t + c] = candidate c of query row t*128 + p
    out_d = nc.declare_dram_parameter("out", [128, N_QT * K_MAX], dt.int32,
                                      isOutput=True)

    with tile.TileContext(nc) as tc, ExitStack() as ctx:
        consts = ctx.enter_context(tc.tile_pool(name="consts", bufs=1))
        vals = ctx.enter_context(tc.tile_pool(name="vals", bufs=3))
        outs = ctx.enter_context(tc.tile_pool(name="outs", bufs=2))
        psum = ctx.enter_context(tc.tile_pool(name="psum", bufs=1, space="PSUM"))

        # ---- load raw (transposed) inputs, split by dim group ----
        # all on SP (the ACT queue would stall a DMA setup behind the
        # activation table load); group 1 first — it gates the first signs
        qraw = consts.tile([D, QSH], dt.bfloat16)
        kraw = consts.tile([D, L], dt.bfloat16)
        nc.sync.dma_start(kraw[0:32, :], kT_d[0:32, :])
        nc.sync.dma_start(qraw[0:32, :], qT_d[0:32, :])
        nc.sync.dma_start(kraw[32:64, :], kT_d[32:64, :])
        nc.sync.dma_start(qraw[32:64, :], qT_d[32:64, :])

        # constants: all memsets/small DMAs on the otherwise-idle Pool
        # engine, ordered by when they are first needed (z64 first: it
        # feeds the PE warmup; the ramp DMAs gate the first matmuls so
        # they precede the slower ones-row memsets)
        z64 = consts.tile([128, 64], dt.float32, tag="z64")
        nc.gpsimd.memset(z64[:], 0.0)
        bias8 = consts.tile([128, 1], dt.float32, tag="bias8")
        nc.gpsimd.memset(bias8[:], -8.0)

        # ACT warmup: a dependency-free activation so the ~1.3us Relu
        # table load happens at t~0 instead of attaching itself (and its
        # successor's waits) to the first real evacuation.
        actw = consts.tile([128, 1], dt.float32, tag="actw")
        nc.scalar.activation(
            actw[:], z64[:, 0:1], mybir.ActivationFunctionType.Relu,
            bias=0.0, scale=1.0)

        # ---- sign tiles (+ ramp rows) ----
        # QS[g]: [34, QSH]  rows 0:32 = query signs (+-0.5), rows 32/33 = 1.0
        # KS[g]: [34, L]    rows 0:32 = key signs (+-0.5), rows 32/33 = ramp
        # All four sign passes on DVE: bf16 in/out, SBUF, step 1 -> 4x mode.
        # (x>0) - 0.5 -> +-0.5 exactly; x == 0 -> -0.5 like the reference.
        QS = []
        KS = []
        for g in range(2):
            qs = consts.tile([34, QSH], dt.bfloat16, tag=f"qs{g}")
            ks = consts.tile([34, L], dt.bfloat16, tag=f"ks{g}")
            QS.append(qs)
            KS.append(ks)
            # ramp terms (host-precomputed bf16 constants) into rows 32/33
            nc.gpsimd.dma_start(ks[32:34, :], ramp_d[:])
        for g in range(2):
            nc.gpsimd.memset(QS[g][32:34, :], 1.0)
        # key signs first: they are the longer pass and gate the first
        # matmul together with the query signs
        for g in range(2):
            lo, hi = g * 32, (g + 1) * 32
            nc.vector.tensor_scalar(
                out=KS[g][0:32, :], in0=kraw[lo:hi, :],
                scalar1=0.0, scalar2=0.5, op0=Alu.is_gt, op1=Alu.subtract)
            nc.vector.tensor_scalar(
                out=QS[g][0:32, :], in0=qraw[lo:hi, :],
                scalar1=0.0, scalar2=0.5, op0=Alu.is_gt, op1=Alu.subtract)

        c2048 = consts.tile([128, 64], dt.float32, tag="c2048")
        nc.gpsimd.memset(c2048[:], 2048.0)
        # all 8 query tiles' top-8 values, decoded per 4-tile half
        t8all = consts.tile([128, 64], dt.float16, tag="t8all")
        # per-tile exact sums: col 2t = Pool accum of the merged h0 block,
        # col 2t+1 = DVE accum of the merged h1 block (both exact: a
        # both-group match contributes once, like the reference's union)
        # (+2 spare cols: tile 0 splits its h0 evacuation for an earlier
        # pipeline start, so its h0 sum arrives as two accums)
        svall = consts.tile([128, 2 * N_QT + 2], dt.float32, tag="svall")
        # packed output block; -1 everywhere the decode doesn't overwrite
        o2 = consts.tile([128, N_QT * K_MAX], dt.int32, tag="o2")
        nc.gpsimd.memset(o2[:], -1)

        # ---- PE warmup: ~10 zero matmuls so the HAM clock gate reaches
        # full speed before the first real matmuls (the activity window is
        # ~3.4us; these run while the DMAs and signs are still in flight).
        # z64 is all zeros so the scratch PSUM results are finite; they
        # reuse the p01 storage, which tile 0 overwrites with start=True.
        pwarm = psum.tile([128, 1024], dt.float32, tag="p0b")
        for w in range(10):
            nc.tensor.matmul(
                pwarm[0:64, w * 64:w * 64 + 64],
                z64[:], z64[:, 0:64], start=True, stop=True)

        t8v = t8all.rearrange("p (t c) -> p t c", c=8)
        sv2 = svall[:, 0:2 * N_QT].rearrange("p (t c) -> p t c", c=2)
        o2v = o2.rearrange("p (t c) -> p t c", c=K_MAX)

        def tail(t0, nt, extra_accum=False):
            """Flags + decode + writeback for tiles t0..t0+nt-1."""
            ts = slice(t0, t0 + nt)
            # sum(val) from the two exact merged-block accums
            up4 = outs.tile([128, 4], dt.float32, tag="up4")
            nc.vector.tensor_tensor(
                out=up4[:, 0:nt], in0=sv2[:, ts, 0], in1=sv2[:, ts, 1],
                op=Alu.add)
            if extra_accum:
                # tile 0's second h0 accum (split evacuation)
                nc.vector.tensor_tensor(
                    out=up4[:, 0:1], in0=up4[:, 0:1],
                    in1=svall[:, 2 * N_QT:2 * N_QT + 1], op=Alu.add)
            # sum of the extracted top-8 values (exact in fp32)
            ts4 = outs.tile([128, 4], dt.float32, tag="ts4")
            nc.vector.tensor_reduce(
                out=ts4[:, 0:nt], in_=t8v[:, ts, :], axis=mybir.AxisListType.X,
                op=Alu.add)
            # flag = some match was dropped (fold collision, > 8 matches,
            # or a 2^-64 both-group match). Written straight into column 63
            # of each tile's output block (the device otherwise always
            # leaves it -1): the host reads it there, so the decode below
            # no longer depends on the flag chain and runs in parallel.
            nc.vector.tensor_tensor(
                out=o2v[:, ts, 63], in0=up4[:, 0:nt], in1=ts4[:, 0:nt],
                op=Alu.is_gt)
            # decode: matched v = (2048-j)*2^-13 => u = 2048 - 8192*v = j;
            # unmatched v = 0 => u = 2048 -> -1.
            w = 8 * nt
            cols = slice(8 * t0, 8 * t0 + w)
            u = outs.tile([128, 32], dt.float32, tag="u")
            nc.vector.scalar_tensor_tensor(
                out=u[:, 0:w], in0=t8all[:, cols], scalar=-8192.0,
                in1=c2048[:, 0:w], op0=Alu.mult, op1=Alu.add)
            pad = outs.tile([128, 32], dt.float32, tag="pad")
            nc.vector.scalar_tensor_tensor(
                out=pad[:, 0:w], in0=u[:, 0:w], scalar=-2047.0,
                in1=z64[:, 0:w], op0=Alu.add, op1=Alu.max)
            # o = u - 2049*pad -> j or -1 (int32 cast on write), scattered
            # into the first 8 columns of each tile's 64-column block
            nc.vector.scalar_tensor_tensor(
                out=o2v[:, ts, 0:8],
                in0=pad[:, 0:w].rearrange("p (t c) -> p t c", c=8),
                scalar=-2049.0,
                in1=u[:, 0:w].rearrange("p (t c) -> p t c", c=8),
                op0=Alu.mult, op1=Alu.add)
            # writeback from SP (idle by now; cheaper DGE setup than SWDGE)
            nc.sync.dma_start(out_d[:, 64 * t0:64 * (t0 + nt)],
                              o2[:, 64 * t0:64 * (t0 + nt)])

        # ---- main loop over query tiles ----
        # reps>1 repeats the whole body inside one NEFF (timing only).
        for _ in range(reps):
            for t in range(N_QT):
                val = vals.tile([128, 1024], dt.float16, tag="val")
                vh0 = vals.tile([128, 2048], dt.float16, tag="vh0")
                v2h1 = vals.tile([128, 1024], dt.float16, tag="v2h1")

                # half 0: both groups' matmuls land in one 4-bank PSUM tile
                # so ONE 2048-wide ACT relu evacuates both; its accumulator
                # gives sum(vh0) >= sum(val_h0), tight unless a 2^-64
                # both-group match (which then just false-positives the
                # exact host fallback). DVE merges with one all-fp16 2x tt.
                p01 = psum.tile([128, 2048], dt.float32, tag="p01")
                for g in range(2):
                    for n in range(2):
                        nc.tensor.matmul(
                            p01[:, g * 1024 + n * 512:g * 1024 + (n + 1) * 512],
                            QS[g][:, t * 128:(t + 1) * 128],
                            KS[g][:, n * 512:(n + 1) * 512],
                            start=True, stop=True)
                if t == 0:
                    # tile 0 only: split the wide evacuation so ACT starts
                    # right after the g1 matmuls instead of waiting for all
                    # four chunks (the two accums are summed in the tail)
                    nc.scalar.activation(
                        vh0[:, 0:1024], p01[:, 0:1024],
                        mybir.ActivationFunctionType.Relu,
                        bias=bias8[:], scale=1.0,
                        accum_out=sv2[:, t, 0:1])
                    nc.scalar.activation(
                        vh0[:, 1024:2048], p01[:, 1024:2048],
                        mybir.ActivationFunctionType.Relu,
                        bias=bias8[:], scale=1.0,
                        accum_out=svall[:, 2 * N_QT:2 * N_QT + 1])
                else:
                    nc.scalar.activation(
                        vh0[:], p01[:], mybir.ActivationFunctionType.Relu,
                        bias=bias8[:], scale=1.0,
                        accum_out=sv2[:, t, 0:1])
                nc.vector.tensor_tensor(
                    out=val[:, 0:1024], in0=vh0[:, 0:1024],
                    in1=vh0[:, 1024:2048], op=Alu.max)

                # half 1: ACT evacuates group 2 (own PSUM tile), DVE
                # evacuates group 1 fused with the merge (PSUM source, 1x)
                # + exact accum; separate tiles keep the two loops
                # independent.
                p0b = psum.tile([128, 1024], dt.float32, tag="p0b")
                p1b = psum.tile([128, 1024], dt.float32, tag="p1b")
                for g, pg in ((1, p1b), (0, p0b)):
                    for n in range(2):
                        nc.tensor.matmul(
                            pg[:, n * 512:(n + 1) * 512],
                            QS[g][:, t * 128:(t + 1) * 128],
                            KS[g][:, 1024 + n * 512:1024 + (n + 1) * 512],
                            start=True, stop=True)
                nc.scalar.activation(
                    v2h1[:], p1b[:], mybir.ActivationFunctionType.Relu,
                    bias=bias8[:], scale=1.0)
                val1 = vals.tile([128, 1024], dt.float16, tag="val1")
                nc.vector.scalar_tensor_tensor(
                    out=val1[:], in0=p0b[:], scalar=-8.0,
                    in1=v2h1[:], op0=Alu.add, op1=Alu.max,
                    accum_out=sv2[:, t, 1:2])

                # fold 2048 -> 256 (all-fp16 2x tt) and extract the top-8
                m1 = vals.tile([128, 1024], dt.float16, tag="m1")
                nc.vector.tensor_tensor(
                    out=m1[:], in0=val[:, 0:1024], in1=val1[:], op=Alu.max)
                m2 = vals.tile([128, 512], dt.float16, tag="m2")
                nc.vector.tensor_tensor(
                    out=m2[:], in0=m1[:, 0:512], in1=m1[:, 512:1024],
                    op=Alu.max)
                m3 = vals.tile([128, 256], dt.float16, tag="m3")
                nc.vector.tensor_tensor(
                    out=m3[:], in0=m2[:, 0:256], in1=m2[:, 256:512],
                    op=Alu.max)
                nc.vector.max(t8all[:, 8 * t:8 * t + 8], m3[:])

                if t == 3:
                    tail(0, 4, extra_accum=True)
                elif t == 6:
                    tail(4, 3)
            tail(7, 1)

    return nc


def _get_program():
    if "prog" not in _CACHE:
        nc = _build_program()
        if not nc.is_finalized():
            nc.finalize()  # Bacc: runs wait-splitting + reg-alloc passes
        _CACHE["prog"] = nc
    return _CACHE["prog"]


def _ramp_rows():
    """[2, L] bf16 rows summing (via the all-ones weight rows) to
    ramp(j) = (2048-j)*2^-13: hi = (128-(j>>4))*2^-9, lo = -(j&15)*2^-13.
    Every term is exactly representable in bf16, and relu(P-16) lands in
    (0, 0.25] where fp16 spacing is <= 2^-13, so values stay exact."""
    import ml_dtypes
    j = np.arange(L)
    hi = (128 - (j >> 4)).astype(np.float32) * 2.0 ** -9
    lo = -(j & 15).astype(np.float32) * 2.0 ** -13
    return np.stack([hi, lo]).astype(ml_dtypes.bfloat16)


def _make_in_maps(q, k):
    import ml_dtypes
    ramp = _ramp_rows()
    in_maps = []
    for c in range(N_CORES):
        b, h = divmod(c, 2)
        # bf16 rounding preserves (x > 0) for all reachable randn fp32
        qT = np.ascontiguousarray(
            q[b, h * QSH:(h + 1) * QSH, :].T.astype(ml_dtypes.bfloat16))
        kT = np.ascontiguousarray(k[b].T.astype(ml_dtypes.bfloat16))
        in_maps.append({"qT": qT, "kT": kT, "ramp": ramp})
    return in_maps


def run_device(q, k, trace=False):
    """Run the bass kernel on the 8 cores.

    Returns (full_out, any_loss_flag): column 63 of each device block
    carries the per-row sum-conservation flag (1 = a match was dropped by
    a fold collision / >8 matches / 2^-64 both-group match); it is read
    out and restored to the -1 padding the reference expects.
    """
    from concourse.bass_utils import run_bass_kernel_spmd

    res = run_bass_kernel_spmd(
        _get_program(), _make_in_maps(q, k), list(range(N_CORES)), trace=trace)
    full = np.empty((B, L, K_MAX), np.int32)
    for c in range(N_CORES):
        b, h = divmod(c, 2)
        # out[p, 64t+c] = result for query row t*128+p
        blk = res.results[c]["out"].reshape(128, N_QT, K_MAX)
        full[b, h * QSH:(h + 1) * QSH, :] = (
            blk.transpose(1, 0, 2).reshape(QSH, K_MAX))
    flagged = bool((full[..., 63] == 1).any())
    full[..., 63] = -1
    return full, flagged


def _reference_numpy(q, k):
    """Exact numpy fallback (used only if some row has >= 8 matches)."""
    out = np.full((B, L, K_MAX), -1, np.int32)
    for b in range(B):
        qb = (q[b] > 0)
        kb = (k[b] > 0)
        match = np.zeros((L, L), bool)
        for lo in (0, 32):
            qg = qb[:, lo:lo + 32]
            kg = kb[:, lo:lo + 32]
            # pack 32 bits into one uint32 per row for exact equality
            qc = np.packbits(qg, axis=1).view(">u4").ravel()
            kc = np.packbits(kg, axis=1).view(">u4").ravel()
            match |= qc[:, None] == kc[None, :]
        for i in range(L):
            idx = np.nonzero(match[i])[0][:K_MAX]
            out[b, i, :len(idx)] = idx
    return out


def kernel(query_up, key_up, head_idx=None, **_unused):
    q = np.asarray(query_up, dtype=np.float32)
    k = np.asarray(key_up, dtype=np.float32)
    assert q.shape == (B, L, D) and k.shape == (B, L, D)
    full, flagged = run_device(q, k)
    # Exact overflow detection: a non(-1) 8th candidate means the row had
    # >= 8 matches (candidates 9.. might have been dropped); the device
    # flag covers fold collisions below that threshold.
    if flagged or (full[..., 7] != -1).any():
        full = _reference_numpy(q, k)
    return full
